# revision 35
# baseline (speedup 1.0000x reference)
"""TRN2 Bass kernel for nn_CaT_36893769073058 (sparse DAG attention, 4 layers).

Contract: kernel(**inputs) takes FULL unsharded inputs (numpy), returns FULL
(16, 512, 256) float32 output. Internally: data-parallel over batch across the
8 NeuronCores (2 batch elements per core), weights/dag replicated.

Math per layer (reference.py):
  K/Q/V = swish(X @ W?)                    (biases are structurally zero)
  T  = dT @ (Q K^T) = (Q d)^T K            [associativity trick]
  spm = (T + dneg) / 8                     [ttr: mask+scale+rowmax in one pass]
  E  = exp(spm - rowmax), ssum = rowsum(E) [Act, accum_out]
  s  = alive / ssum                        (dead rows -> s = 0 -> P row = 0)
  Ppl^T = (E diag(s))^T + d               [diag(s) fused into the PE transpose,
                                            +d fused into the PSUM->SBUF copy]
  O  = Ppl @ V                             (single pass: P@V + dT@V fused)
  mha = swish(O @ Wp);  X' = mha + swish(mha @ W1) @ W2
Final: X @ Wlm.

Layout: X transposed (feature-on-partition, token-on-free). Q/V computed in
natural (token-on-partition) layout via lhsT = X^T tiles - no Q/V transposes.
Logit-path matmuls in float32r; E-transposes and the O path in bf16
(PE transposes run at 1.0 cycles/row vs 1.5 for fp32r). Activations are
batched by function (Silu<->Exp act-table switch costs 1.28us). Elementwise
work is load-balanced across DVE and the Pool(gpsimd) engine.
"""

import sys
import types
from contextlib import ExitStack

sys.path.insert(0, "/opt/trn_rl_repo")

import numpy as np

import concourse.bass as bass  # noqa: F401
import concourse.tile as tile
from concourse import bacc, mybir

F32 = mybir.dt.float32
F32R = mybir.dt.float32r
BF16 = mybir.dt.bfloat16
AFT = mybir.ActivationFunctionType
ALU = mybir.AluOpType
AX = mybir.AxisListType

B, N, D = 16, 512, 256
L, H, HS, FF = 4, 8, 64, 1024
NCORES = 8
BPC = B // NCORES
NC4 = N // 128             # 4
DC = D // 128              # 2
FC = FF // 128             # 8
NEG_BIG = 1.5625e29        # additive mask (post 1/8 scale)
L0_SHIFT = 75.0            # layer-0 constant exp shift (|spm| <= 143 checked)
L0_FAST = False            # layer-0 constant-shift fast path
LOOKAHEAD = 3
USE_POOL = False           # gpsimd too slow in practice; keep DVE
TTR_INPLACE = True         # ttr writes spm back into T psum (bisect flag)


def _install_ntff_hook():
    """Recreate the missing antenv.axon_hooks so trace=True can profile."""
    if "antenv.axon_hooks" in sys.modules:
        return
    try:
        import antenv

        mod = types.ModuleType("antenv.axon_hooks")
        state = {"hook": None}
        mod.set_axon_ntff_profile_hook = lambda h: state.__setitem__("hook", h)
        mod.get_axon_ntff_profile_hook = lambda: state["hook"]
        sys.modules["antenv.axon_hooks"] = mod
        antenv.axon_hooks = mod
        if "/root/.axon_site" not in sys.path:
            sys.path.insert(0, "/root/.axon_site")
        from trn_agent_boot.trn_boot import _ntff_profile_via_ctypes

        mod.set_axon_ntff_profile_hook(
            _ntff_profile_via_ctypes("/opt/axon/libaxon_pjrt.so")
        )
    except Exception:
        pass


def _build():
    nc = bacc.Bacc("TRN2", target_bir_lowering=False, debug=False,
                   num_devices=NCORES)

    def din(name, shape, dt=F32):
        return nc.dram_tensor(name, list(shape), dt, kind="ExternalInput").ap()

    xt_d = din("xt", (BPC, D, N))
    dmat_d = din("dmat", (2, N, N))              # [v][m, i] f32 (QD rhs)
    dnat_bf_d = din("dnat_bf", (2, 2, 128, 1024), BF16)  # [v][th][j%128, jcl*512+i]
    dneg_d = din("dneg", (2, 2, 128, 1024), BF16)  # [v][th][i%128, i2*512+j]
    dtm_d = din("dtm", (2, 128, 1024), BF16)     # [th][i%128, i2*512+j] (v=0 mask)
    eye_bf_d = din("eye_bf", (128, 128), BF16)
    wk_d = din("wk", (L, D, H * HS))
    wq_d = din("wq", (L, D, H * HS))
    wv_d = din("wv", (L, D, H * HS))
    wp_d = din("wp", (L, H * HS, D))
    w1_d = din("w1", (L, D, FF))
    w2_d = din("w2_bf", (L, FF, D), BF16)
    wlm_d = din("wlm", (D, D))
    out_d = nc.dram_tensor("out", [BPC, D, N], F32, kind="ExternalOutput").ap()

    with tile.TileContext(nc) as tc, ExitStack() as ctx:
        # ---------------- pools ----------------
        pconst = ctx.enter_context(tc.tile_pool(name="pconst", bufs=1))
        pw = ctx.enter_context(tc.tile_pool(name="pw", bufs=2))      # kqv weights
        pw1 = ctx.enter_context(tc.tile_pool(name="pw1", bufs=1))    # wp/w1/w2
        pxt = ctx.enter_context(tc.tile_pool(name="pxt", bufs=2))
        pkqv = ctx.enter_context(tc.tile_pool(name="pkqv", bufs=1))  # k/q/v per b
        pqd = ctx.enter_context(tc.tile_pool(name="pqd", bufs=1))
        pem = ctx.enter_context(tc.tile_pool(name="pem", bufs=4))
        psml = ctx.enter_context(tc.tile_pool(name="psml", bufs=3))
        pspm = ctx.enter_context(tc.tile_pool(name="pspm", bufs=2))
        pptsb = ctx.enter_context(tc.tile_pool(name="pptsb", bufs=2))
        po = ctx.enter_context(tc.tile_pool(name="po", bufs=1))
        pmha = ctx.enter_context(tc.tile_pool(name="pmha", bufs=1))
        pff1 = ctx.enter_context(tc.tile_pool(name="pff1", bufs=1))
        pout = ctx.enter_context(tc.tile_pool(name="pout", bufs=1))
        # PSUM: 4 + 2 + 2 = 8 banks
        ps_big = ctx.enter_context(tc.tile_pool(name="ps_big", bufs=2, space="PSUM"))
        ps_pt = ctx.enter_context(tc.tile_pool(name="ps_pt", bufs=2, space="PSUM"))
        ps_sm = ctx.enter_context(tc.tile_pool(name="ps_sm", bufs=2, space="PSUM"))

        # ---------------- static loads ----------------
        eye_bf = pconst.tile([128, 128], BF16, tag="eye", name="eye")
        nc.sync.dma_start(eye_bf[:], eye_bf_d[:])
        l0b = pconst.tile([128, 1], F32, tag="l0b", name="l0b")
        nc.vector.memset(l0b[:], -L0_SHIFT)

        d_r = {}      # [(v, mc)] (128, 512) f32r: d[m, i], rows m-chunk
        dnat = {}     # [(v, th)] (128, 1024) bf16: rows j%128, [jcl*512+i]
        dng = {}
        dtm = {}      # [(v, ic)] (128, 512) f32: additive mask rows i-chunk
        for v in range(2):
            for c in range(NC4):
                t = pconst.tile([128, N], F32R, tag=f"d{v}_{c}", name=f"d{v}_{c}")
                nc.sync.dma_start(t[:], dmat_d[v, c * 128:(c + 1) * 128, :]
                                  .bitcast(F32R))
                d_r[(v, c)] = t

            for th in range(2):
                tn = pconst.tile([128, 1024], BF16, tag=f"dn{v}_{th}",
                                 name=f"dn{v}_{th}")
                nc.sync.dma_start(tn[:], dnat_bf_d[v, th])
                dnat[(v, th)] = tn
                tg = pconst.tile([128, 1024], BF16, tag=f"dg{v}_{th}",
                                 name=f"dg{v}_{th}")
                nc.sync.dma_start(tg[:], dneg_d[v, th])
                dng[(v, th)] = tg
                if v == 0:
                    tq = pconst.tile([128, 1024], BF16, tag=f"dtm{th}",
                                     name=f"dtm{th}")
                    nc.sync.dma_start(tq[:], dtm_d[th])
                    dtm[(0, th)] = tq

        wlm_t = []
        for kc in range(DC):
            t = pconst.tile([128, D], F32R, tag=f"wlm{kc}", name=f"wlm{kc}")
            nc.sync.dma_start(t[:], wlm_d[kc * 128:(kc + 1) * 128, :]
                              .bitcast(F32R))
            wlm_t.append(t)

        xt_cur = {}
        for b in range(BPC):
            tiles = []
            for c in range(DC):
                t = pxt.tile([128, N], F32R, tag=f"xt{b}_{c}", name=f"xt{b}_{c}")
                nc.sync.dma_start(t[:], xt_d[b, c * 128:(c + 1) * 128, :]
                                  .bitcast(F32R))
                tiles.append(t)
            xt_cur[b] = tiles

        # ---------------- layers ----------------
        for l in range(L):
            v = 0 if l == 0 else 1

            wk_t, wq_t, wv_t = [], [], []
            for kc in range(DC):
                for (dst, src, nm) in ((wk_t, wk_d, "wk"), (wq_t, wq_d, "wq"),
                                       (wv_t, wv_d, "wv")):
                    t = pw.tile([128, H * HS], F32R, tag=f"{nm}{kc}", name=nm)
                    nc.sync.dma_start(t[:], src[l, kc * 128:(kc + 1) * 128, :]
                                      .bitcast(F32R))
                    dst.append(t)
            wp_t = []
            for kc in range(4):
                t = pw1.tile([128, D], F32R, tag=f"wp{kc}", name="wp")
                nc.sync.dma_start(t[:], wp_d[l, kc * 128:(kc + 1) * 128, :]
                                  .bitcast(F32R))
                wp_t.append(t)
            w1_t = []
            for kc in range(DC):
                t = pw1.tile([128, FF], F32R, tag=f"w1{kc}", name="w1")
                nc.sync.dma_start(t[:], w1_d[l, kc * 128:(kc + 1) * 128, :]
                                  .bitcast(F32R))
                w1_t.append(t)
            w2_t = []
            for kc in range(FC):
                t = pw1.tile([128, D], BF16, tag=f"w2{kc}", name="w2")
                nc.sync.dma_start(t[:], w2_d[l, kc * 128:(kc + 1) * 128, :])
                w2_t.append(t)

            # ---- KQV phase (both b) ----
            k_sb, q_nat, v_nat = {}, {}, {}
            for b in range(BPC):
                xt = xt_cur[b]
                ks, qs, vs = [], [], []
                for g in range(2):
                    # K pair-packed: (128=2heads*64, m); hp = 2g, 2g+1
                    mm = ps_big.tile([128, 1024], F32, tag="big", name="kps")
                    for i2 in range(2):
                        hp = 2 * g + i2
                        for kc in range(DC):
                            nc.tensor.matmul(
                                mm[:, i2 * 512:(i2 + 1) * 512],
                                wk_t[kc][:, hp * 128:(hp + 1) * 128],
                                xt[kc][:], start=(kc == 0), stop=(kc == DC - 1))
                    sb = pkqv.tile([128, 1024], F32R, tag=f"k{b}{g}", name="ksb")
                    nc.scalar.activation(sb[:], mm[:], AFT.Silu)
                    ks.append(sb)
                for g in range(2):
                    # Q natural: (m-chunk, hk) via lhsT = xt;  mc = 2g, 2g+1
                    mm = ps_big.tile([128, 1024], F32, tag="big", name="qps")
                    for i2 in range(2):
                        mc = 2 * g + i2
                        for kc in range(DC):
                            nc.tensor.matmul(
                                mm[:, i2 * 512:(i2 + 1) * 512],
                                xt[kc][:, mc * 128:(mc + 1) * 128],
                                wq_t[kc][:], start=(kc == 0), stop=(kc == DC - 1))
                    sb = pkqv.tile([128, 1024], F32R, tag=f"q{b}{g}", name="qsb")
                    nc.scalar.activation(sb[:], mm[:], AFT.Silu)
                    qs.append(sb)
                for g in range(2):
                    # V natural (j-chunk, hk), bf16
                    mm = ps_big.tile([128, 1024], F32, tag="big", name="vps")
                    for i2 in range(2):
                        jc = 2 * g + i2
                        for kc in range(DC):
                            nc.tensor.matmul(
                                mm[:, i2 * 512:(i2 + 1) * 512],
                                xt[kc][:, jc * 128:(jc + 1) * 128],
                                wv_t[kc][:], start=(kc == 0), stop=(kc == DC - 1))
                    sb = pkqv.tile([128, 1024], BF16, tag=f"v{b}{g}", name="vsb")
                    nc.scalar.activation(sb[:], mm[:], AFT.Silu)
                    vs.append(sb)
                k_sb[b], q_nat[b], v_nat[b] = ks, qs, vs

            # ---- head loop: b-interleaved, software-pipelined ----
            seq = [(b, h) for h in range(H) for b in range(BPC)]
            qd_sb = {}
            o_all = {b: [None] * 4 for b in range(BPC)}
            o_ps = {}
            state = {}
            ptadd_ctr = [0]

            def emit_qd(b, hp):
                mm = ps_sm.tile([128, N], F32, tag="sm", name="qdps")
                for mc in range(NC4):
                    g, i2 = mc // 2, mc % 2
                    nc.tensor.matmul(
                        mm[:],
                        q_nat[b][g][:, i2 * 512 + hp * 128:
                                    i2 * 512 + (hp + 1) * 128],
                        d_r[(v, mc)][:], start=(mc == 0), stop=(mc == NC4 - 1))
                sb = pqd.tile([128, N], F32R, tag=f"qd{b}", name="qdsb")
                nc.scalar.activation(sb[:], mm[:], AFT.Copy)
                qd_sb[b] = sb

            def stage_front(b, h):
                hp, half = h // 2, h % 2
                lo, hi = half * 64, (half + 1) * 64
                qd = qd_sb[b]
                kg, ki = hp // 2, hp % 2
                ksl = k_sb[b][kg]
                # T matmuls -> 2 psum tiles (128,1024) = [ic0|ic1], [ic2|ic3]
                tts = []
                for th in range(2):
                    tt = ps_big.tile([128, 1024], F32, tag="big", name="tps")
                    for i2 in range(2):
                        ic = 2 * th + i2
                        nc.tensor.matmul(
                            tt[:, i2 * 512:(i2 + 1) * 512],
                            qd[lo:hi, ic * 128:(ic + 1) * 128],
                            ksl[lo:hi, ki * 512:(ki + 1) * 512],
                            start=True, stop=True)
                    tts.append(tt)
                em = pem.tile([128, 2048], BF16, tag="em", name="em")
                ssum = psml.tile([128, 4], F32, tag="ssum", name="ssum")
                s4 = psml.tile([128, 4], F32, tag="s4", name="s4")
                if l == 0 and L0_FAST:
                    # logits bounded (|spm| <= 143 for this problem's data):
                    # constant exp shift, multiplicative mask with masked
                    # row-sums via STT accum. No stt/rowmax passes.
                    for th in range(2):
                        nc.scalar.activation(
                            em[:, th * 1024:(th + 1) * 1024], tts[th][:],
                            AFT.Exp, bias=l0b[:, 0:1], scale=0.125)
                    for ic in range(NC4):
                        th, i2 = ic // 2, ic % 2
                        nc.vector.scalar_tensor_tensor(
                            em[:, ic * 512:(ic + 1) * 512],
                            em[:, ic * 512:(ic + 1) * 512], 1.0,
                            dtm[(0, th)][:, i2 * 512:(i2 + 1) * 512],
                            ALU.mult, ALU.mult,
                            accum_out=ssum[:, ic:ic + 1])
                    al = psml.tile([128, 4], F32, tag="al", name="al")
                    nc.vector.tensor_scalar(al[:], ssum[:], 0.0, None,
                                            ALU.is_le)
                    nc.vector.tensor_tensor(al[:], ssum[:], al[:], ALU.add)
                    nc.vector.reciprocal(s4[:], al[:])
                else:
                    negm = psml.tile([128, 4], F32, tag="negm", name="negm")
                    al = psml.tile([128, 4], F32, tag="al", name="al")
                    for ic in range(NC4):
                        th, i2 = ic // 2, ic % 2
                        spm = pspm.tile([128, 512], F32, tag=f"spm{ic % 2}",
                                        name="spm", bufs=2)
                        nc.vector.scalar_tensor_tensor(
                            spm[:, 0:512],
                            tts[th][:, i2 * 512:(i2 + 1) * 512],
                            0.125, dng[(v, th)][:, i2 * 512:(i2 + 1) * 512],
                            ALU.mult, ALU.add)
                        nc.vector.tensor_reduce(
                            negm[:, ic:ic + 1], spm[:, 0:512], AX.X,
                            ALU.max, negate=True)
                        nc.scalar.activation(
                            em[:, ic * 512:(ic + 1) * 512],
                            spm[:, 0:512], AFT.Exp,
                            bias=negm[:, ic:ic + 1], scale=1.0,
                            accum_out=ssum[:, ic:ic + 1])
                    nc.vector.tensor_scalar(al[:], negm[:], 1.0e28,
                                            None, ALU.is_lt)
                    nc.vector.reciprocal(s4[:], ssum[:])
                    nc.vector.tensor_tensor(s4[:], s4[:], al[:], ALU.mult)
                for ic in range(NC4):
                    nc.vector.tensor_scalar(
                        em[:, ic * 512:(ic + 1) * 512],
                        em[:, ic * 512:(ic + 1) * 512],
                        s4[:, ic:ic + 1], None, ALU.mult)
                state[(b, h)] = (em,)

            def stage_back(b, h):
                hp, half = h // 2, h % 2
                (em,) = state.pop((b, h))
                # transposes (plain identity) -> pt psum (j, i) bf16
                pts = []
                for th in range(2):
                    pts.append(ps_pt.tile([128, 1024], BF16, tag="pt", name="pt"))
                for ic in range(NC4):
                    for jc in range(NC4):
                        nc.tensor.transpose(
                            pts[jc // 2][:, (jc % 2) * 512 + ic * 128:
                                         (jc % 2) * 512 + (ic + 1) * 128],
                            em[:, ic * 512 + jc * 128: ic * 512 + (jc + 1) * 128],
                            eye_bf[:])
                # Ppl^T = pt + d  (psum->sbuf on DVE)
                ptsb = []
                for th in range(2):
                    t = pptsb.tile([128, 1024], BF16, tag=f"ptsb{th}", name="ptsb")
                    nc.vector.tensor_tensor(t[:], pts[th][:], dnat[(v, th)][:],
                                            ALU.add)
                    ptsb.append(t)
                # O = Ppl @ V : accumulate into pair-packed psum
                if half == 0:
                    o_ps[b] = ps_sm.tile([128, N], F32, tag="sm", name="ops")
                lo = half * 64
                vg = v_nat[b]
                for jc in range(NC4):
                    nc.tensor.matmul(
                        o_ps[b][lo:lo + 64, :],
                        vg[jc // 2][:, (jc % 2) * 512 + h * 64:
                                    (jc % 2) * 512 + (h + 1) * 64],
                        ptsb[jc // 2][:, (jc % 2) * 512:(jc % 2 + 1) * 512],
                        start=(jc == 0), stop=(jc == NC4 - 1))
                if half == 1:
                    ot = po.tile([128, N], F32R, tag=f"o{b}{hp}", name="oall")
                    nc.scalar.activation(ot[:], o_ps[b][:], AFT.Copy)
                    o_all[b][hp] = ot

            for i in range(len(seq) + LOOKAHEAD):
                if i >= LOOKAHEAD:
                    stage_back(*seq[i - LOOKAHEAD])
                if i < len(seq):
                    b, h = seq[i]
                    if h % 2 == 0:
                        emit_qd(b, h // 2)
                    stage_front(b, h)

            # ---- MLP (per b) ----
            for b in range(BPC):
                mm = ps_big.tile([128, 1024], F32, tag="big", name="mhaps")
                for mc in range(DC):
                    for kc in range(4):
                        nc.tensor.matmul(
                            mm[:, mc * 512:(mc + 1) * 512],
                            wp_t[kc][:, mc * 128:(mc + 1) * 128],
                            o_all[b][kc][:], start=(kc == 0), stop=(kc == 3))
                mha = pmha.tile([128, 1024], F32R, tag=f"mha{b}", name="mha")
                nc.scalar.activation(mha[:], mm[:], AFT.Silu)

                ff1 = []
                for g in range(4):  # fc pairs
                    mm = ps_big.tile([128, 1024], F32, tag="big", name="ff1ps")
                    for i2 in range(2):
                        fc = 2 * g + i2
                        for mc in range(DC):
                            nc.tensor.matmul(
                                mm[:, i2 * 512:(i2 + 1) * 512],
                                w1_t[mc][:, fc * 128:(fc + 1) * 128],
                                mha[:, mc * 512:(mc + 1) * 512],
                                start=(mc == 0), stop=(mc == DC - 1))
                    t = pff1.tile([128, 1024], BF16, tag=f"ff1{g}", name="ff1")
                    nc.scalar.activation(t[:], mm[:], AFT.Silu)
                    ff1.append(t)

                xt_new = []
                for mc in range(DC):
                    mm = ps_sm.tile([128, N], F32, tag="sm", name="ff2ps")
                    for fc in range(FC):
                        nc.tensor.matmul(
                            mm[:], w2_t[fc][:, mc * 128:(mc + 1) * 128],
                            ff1[fc // 2][:, (fc % 2) * 512:(fc % 2 + 1) * 512],
                            start=(fc == 0), stop=(fc == FC - 1))
                    t = pxt.tile([128, N], F32R, tag=f"xt{b}_{mc}",
                                 name=f"xt{b}_{mc}")
                    nc.vector.tensor_tensor(t[:], mm[:],
                                            mha[:, mc * 512:(mc + 1) * 512],
                                            ALU.add)
                    xt_new.append(t)
                xt_cur[b] = xt_new

        # ---------------- lm head ----------------
        for b in range(BPC):
            for mc in range(DC):
                mm = ps_sm.tile([128, N], F32, tag="sm", name="lmps")
                for kc in range(DC):
                    nc.tensor.matmul(
                        mm[:], wlm_t[kc][:, mc * 128:(mc + 1) * 128],
                        xt_cur[b][kc][:], start=(kc == 0), stop=(kc == DC - 1))
                ot = pout.tile([128, N], F32, tag="out", name="out")
                nc.scalar.activation(ot[:], mm[:], AFT.Copy)
                nc.sync.dma_start(out_d[b, mc * 128:(mc + 1) * 128, :], ot[:])

    nc.compile()
    return nc


_NC_CACHE = None


def _get_nc():
    global _NC_CACHE
    if _NC_CACHE is None:
        _NC_CACHE = _build()
    return _NC_CACHE


def _bf16_np(x):
    import ml_dtypes
    return np.ascontiguousarray(
        np.asarray(x, dtype=np.float32).astype(ml_dtypes.bfloat16))


def _prep_inputs(inputs):
    f = lambda x: np.ascontiguousarray(np.asarray(x, dtype=np.float32))
    for bn in ("bk", "bq", "bv", "bp", "b1", "b2", "blm"):
        if np.any(np.asarray(inputs[bn]) != 0):
            raise ValueError(f"kernel compiled for zero {bn}")
    X = f(inputs["X"])
    dag = np.asarray(inputs["dag"])
    d0 = np.clip(dag.astype(np.float32), 0.0, 1.0)
    d1 = np.clip(d0 + np.eye(N, dtype=np.float32), 0.0, 1.0)
    dmat = np.stack([d0, d1])                          # [v][m, i]
    # [v][th][j%128, jcl*512 + i]
    dnat_bf = _bf16_np(dmat.reshape(2, 2, 2, 128, N).transpose(0, 1, 3, 2, 4)
                       .reshape(2, 2, 128, 1024))
    dnegf = (dmat.transpose(0, 2, 1) - 1.0) * NEG_BIG      # [v][i, j]
    dneg = _bf16_np(dnegf.reshape(2, 2, 2, 128, N).transpose(0, 1, 3, 2, 4)
                    .reshape(2, 2, 128, 1024))
    dtm = _bf16_np(dmat[0].T.reshape(2, 2, 128, N).transpose(0, 2, 1, 3)
                   .reshape(2, 128, 1024))
    wr = lambda w: np.ascontiguousarray(
        f(w).transpose(0, 2, 1, 3).reshape(L, D, H * HS))
    common = {
        "dmat": np.ascontiguousarray(dmat),
        "dnat_bf": dnat_bf,
        "dneg": dneg,
        "dtm": dtm,
        "eye_bf": _bf16_np(np.eye(128, dtype=np.float32)),
        "wk": wr(inputs["Wk"]), "wq": wr(inputs["Wq"]), "wv": wr(inputs["Wv"]),
        "wp": f(inputs["Wp"]),
        "w1": f(inputs["W1"]),
        "w2_bf": _bf16_np(inputs["W2"]),
        "wlm": f(inputs["Wlm"]),
    }
    xt_full = np.ascontiguousarray(X.transpose(0, 2, 1))   # (B, D, N)
    in_maps = []
    for c in range(NCORES):
        m = dict(common)
        m["xt"] = np.ascontiguousarray(xt_full[c * BPC:(c + 1) * BPC])
        in_maps.append(m)
    return in_maps


def run(inputs, trace=False):
    from concourse.bass_utils import run_bass_kernel_spmd

    if trace:
        _install_ntff_hook()
    nc = _get_nc()
    in_maps = _prep_inputs(inputs)
    res = run_bass_kernel_spmd(nc, in_maps, list(range(NCORES)), trace=trace)
    outs = np.concatenate([res.results[c]["out"] for c in range(NCORES)], 0)
    full = np.ascontiguousarray(outs.transpose(0, 2, 1).astype(np.float32))
    return full, res


def kernel(**inputs):
    out, _ = run(inputs, trace=False)
    return out


if __name__ == "__main__":
    rng = np.random.default_rng(0)
    fake = {
        "X": rng.standard_normal((B, N, D), dtype=np.float32),
        "dag": rng.integers(0, 2, (N, N)).astype(np.int32),
        "Wk": rng.standard_normal((L, H, D, HS), dtype=np.float32) * 0.05,
        "bk": np.zeros((L, H, HS), np.float32),
        "Wq": rng.standard_normal((L, H, D, HS), dtype=np.float32) * 0.05,
        "bq": np.zeros((L, H, HS), np.float32),
        "Wv": rng.standard_normal((L, H, D, HS), dtype=np.float32) * 0.05,
        "bv": np.zeros((L, H, HS), np.float32),
        "Wp": rng.standard_normal((L, H * HS, D), dtype=np.float32) * 0.05,
        "bp": np.zeros((L, D), np.float32),
        "W1": rng.standard_normal((L, D, FF), dtype=np.float32) * 0.05,
        "b1": np.zeros((L, FF), np.float32),
        "W2": rng.standard_normal((L, FF, D), dtype=np.float32) * 0.05,
        "b2": np.zeros((L, D), np.float32),
        "Wlm": rng.standard_normal((D, D), dtype=np.float32) * 0.05,
        "blm": np.zeros((D,), np.float32),
    }
    out = kernel(**fake)
    print("out", out.shape, out.dtype, np.abs(out).mean())


# revision 36
# speedup vs baseline: 1.0350x; 1.0350x over previous
"""TRN2 Bass kernel for nn_CaT_36893769073058 (sparse DAG attention, 4 layers).

Contract: kernel(**inputs) takes FULL unsharded inputs (numpy), returns FULL
(16, 512, 256) float32 output. Internally: data-parallel over batch across the
8 NeuronCores (2 batch elements per core), weights/dag replicated.

Math per layer (reference.py):
  K/Q/V = swish(X @ W?)                  (biases are structurally zero)
  T   = dT @ (Q K^T) = (Q d)^T K         [associativity: n^3 -> n^2*hs]
  spm = T/8 + dneg                       [additive mask, DVE stt]
  E   = exp(spm - rowmax(spm)) bf16      [rowmax DVE; exp+rowsum Act accum]
  P   = E * (alive/ssum)                 (dead rows -> 0; in-place DVE scale)
  Ppl^T = P^T + d                        [+d fused into the PSUM->SBUF copy,
                                          P^T via bf16 PE transposes]
  O   = Ppl @ V                          (single pass: P@V + dT@V fused)
  mha = swish(O @ Wp);  X' = mha + swish(mha @ W1) @ W2
Final: X @ Wlm.

Layout: X transposed (feature-on-partition, token-on-free). Q and V are
computed in natural (token-on-partition) layout via lhsT = X^T tiles, which
eliminates the Q/V transposes entirely. Logit-path matmuls run in float32r;
the E-transposes and O path run in bf16 (PE transposes: 1.0 cycles/row vs
1.5 for fp32r). Activations are batched by function across a layer
(Silu<->Exp act-table switch costs 1.28us). The head loop interleaves the
two batch elements and is software-pipelined (LOOKAHEAD) so the tensor
engine streams T/transpose/O matmuls of different heads while DVE/Act work
through the softmax chain of earlier heads.
"""

import sys
import types
from contextlib import ExitStack

sys.path.insert(0, "/opt/trn_rl_repo")

import numpy as np

import concourse.bass as bass  # noqa: F401
import concourse.tile as tile
from concourse import bacc, mybir

F32 = mybir.dt.float32
F32R = mybir.dt.float32r
BF16 = mybir.dt.bfloat16
AFT = mybir.ActivationFunctionType
ALU = mybir.AluOpType
AX = mybir.AxisListType

B, N, D = 16, 512, 256
L, H, HS, FF = 4, 8, 64, 1024
NCORES = 8
BPC = B // NCORES
NC4 = N // 128             # 4
DC = D // 128              # 2
FC = FF // 128             # 8
NEG_BIG = 1.5625e29        # additive mask (post 1/8 scale)
L0_SHIFT = 75.0            # layer-0 constant exp shift (|spm| <= 143 checked)
L0_FAST = False            # layer-0 constant-shift fast path
LOOKAHEAD = 2
USE_POOL = False           # gpsimd too slow in practice; keep DVE
TTR_INPLACE = True         # ttr writes spm back into T psum (bisect flag)


def _install_ntff_hook():
    """Recreate the missing antenv.axon_hooks so trace=True can profile."""
    if "antenv.axon_hooks" in sys.modules:
        return
    try:
        import antenv

        mod = types.ModuleType("antenv.axon_hooks")
        state = {"hook": None}
        mod.set_axon_ntff_profile_hook = lambda h: state.__setitem__("hook", h)
        mod.get_axon_ntff_profile_hook = lambda: state["hook"]
        sys.modules["antenv.axon_hooks"] = mod
        antenv.axon_hooks = mod
        if "/root/.axon_site" not in sys.path:
            sys.path.insert(0, "/root/.axon_site")
        from trn_agent_boot.trn_boot import _ntff_profile_via_ctypes

        mod.set_axon_ntff_profile_hook(
            _ntff_profile_via_ctypes("/opt/axon/libaxon_pjrt.so")
        )
    except Exception:
        pass


def _build():
    nc = bacc.Bacc("TRN2", target_bir_lowering=False, debug=False,
                   num_devices=NCORES)

    def din(name, shape, dt=F32):
        return nc.dram_tensor(name, list(shape), dt, kind="ExternalInput").ap()

    xt_d = din("xt", (BPC, D, N))
    dmat_d = din("dmat", (2, N, N))              # [v][m, i] f32 (QD rhs)
    dnat_bf_d = din("dnat_bf", (2, 2, 128, 1024), BF16)  # [v][th][j%128, jcl*512+i]
    dneg_d = din("dneg", (2, 2, 128, 1024), BF16)  # [v][th][i%128, i2*512+j]
    dtm_d = din("dtm", (2, 128, 1024), BF16)     # [th][i%128, i2*512+j] (v=0 mask)
    eye_bf_d = din("eye_bf", (128, 128), BF16)
    wk_d = din("wk", (L, D, H * HS))
    wq_d = din("wq", (L, D, H * HS))
    wv_d = din("wv", (L, D, H * HS))
    wp_d = din("wp", (L, H * HS, D))
    w1_d = din("w1", (L, D, FF))
    w2_d = din("w2_bf", (L, FF, D), BF16)
    wlm_d = din("wlm", (D, D))
    out_d = nc.dram_tensor("out", [BPC, D, N], F32, kind="ExternalOutput").ap()

    with tile.TileContext(nc) as tc, ExitStack() as ctx:
        # ---------------- pools ----------------
        pconst = ctx.enter_context(tc.tile_pool(name="pconst", bufs=1))
        pw = ctx.enter_context(tc.tile_pool(name="pw", bufs=2))      # kqv weights
        pw1 = ctx.enter_context(tc.tile_pool(name="pw1", bufs=1))    # wp/w1/w2
        pxt = ctx.enter_context(tc.tile_pool(name="pxt", bufs=2))
        pkqv = ctx.enter_context(tc.tile_pool(name="pkqv", bufs=1))  # k/q/v per b
        pqd = ctx.enter_context(tc.tile_pool(name="pqd", bufs=1))
        pem = ctx.enter_context(tc.tile_pool(name="pem", bufs=3))
        psml = ctx.enter_context(tc.tile_pool(name="psml", bufs=3))
        pspm = ctx.enter_context(tc.tile_pool(name="pspm", bufs=2))
        pptsb = ctx.enter_context(tc.tile_pool(name="pptsb", bufs=2))
        po = ctx.enter_context(tc.tile_pool(name="po", bufs=1))
        pmha = ctx.enter_context(tc.tile_pool(name="pmha", bufs=1))
        pff1 = ctx.enter_context(tc.tile_pool(name="pff1", bufs=1))
        pout = ctx.enter_context(tc.tile_pool(name="pout", bufs=1))
        # PSUM: 4 + 2 + 2 = 8 banks
        ps_big = ctx.enter_context(tc.tile_pool(name="ps_big", bufs=2, space="PSUM"))
        ps_pt = ctx.enter_context(tc.tile_pool(name="ps_pt", bufs=2, space="PSUM"))
        ps_sm = ctx.enter_context(tc.tile_pool(name="ps_sm", bufs=2, space="PSUM"))

        # ---------------- static loads ----------------
        eye_bf = pconst.tile([128, 128], BF16, tag="eye", name="eye")
        nc.sync.dma_start(eye_bf[:], eye_bf_d[:])
        l0b = pconst.tile([128, 1], F32, tag="l0b", name="l0b")
        nc.vector.memset(l0b[:], -L0_SHIFT)

        d_r = {}      # [(v, mc)] (128, 512) f32r: d[m, i], rows m-chunk
        dnat = {}     # [(v, th)] (128, 1024) bf16: rows j%128, [jcl*512+i]
        dng = {}
        dtm = {}      # [(v, ic)] (128, 512) f32: additive mask rows i-chunk
        for v in range(2):
            for c in range(NC4):
                t = pconst.tile([128, N], F32R, tag=f"d{v}_{c}", name=f"d{v}_{c}")
                nc.sync.dma_start(t[:], dmat_d[v, c * 128:(c + 1) * 128, :]
                                  .bitcast(F32R))
                d_r[(v, c)] = t

            for th in range(2):
                tn = pconst.tile([128, 1024], BF16, tag=f"dn{v}_{th}",
                                 name=f"dn{v}_{th}")
                nc.sync.dma_start(tn[:], dnat_bf_d[v, th])
                dnat[(v, th)] = tn
                tg = pconst.tile([128, 1024], BF16, tag=f"dg{v}_{th}",
                                 name=f"dg{v}_{th}")
                nc.sync.dma_start(tg[:], dneg_d[v, th])
                dng[(v, th)] = tg
                if v == 0:
                    tq = pconst.tile([128, 1024], BF16, tag=f"dtm{th}",
                                     name=f"dtm{th}")
                    nc.sync.dma_start(tq[:], dtm_d[th])
                    dtm[(0, th)] = tq

        wlm_t = []
        for kc in range(DC):
            t = pconst.tile([128, D], F32R, tag=f"wlm{kc}", name=f"wlm{kc}")
            nc.sync.dma_start(t[:], wlm_d[kc * 128:(kc + 1) * 128, :]
                              .bitcast(F32R))
            wlm_t.append(t)

        xt_cur = {}
        for b in range(BPC):
            tiles = []
            for c in range(DC):
                t = pxt.tile([128, N], F32R, tag=f"xt{b}_{c}", name=f"xt{b}_{c}")
                nc.sync.dma_start(t[:], xt_d[b, c * 128:(c + 1) * 128, :]
                                  .bitcast(F32R))
                tiles.append(t)
            xt_cur[b] = tiles

        # ---------------- layers ----------------
        for l in range(L):
            v = 0 if l == 0 else 1

            wk_t, wq_t, wv_t = [], [], []
            for kc in range(DC):
                for (dst, src, nm) in ((wk_t, wk_d, "wk"), (wq_t, wq_d, "wq"),
                                       (wv_t, wv_d, "wv")):
                    t = pw.tile([128, H * HS], F32R, tag=f"{nm}{kc}", name=nm)
                    nc.sync.dma_start(t[:], src[l, kc * 128:(kc + 1) * 128, :]
                                      .bitcast(F32R))
                    dst.append(t)
            wp_t = []
            for kc in range(4):
                t = pw1.tile([128, D], F32R, tag=f"wp{kc}", name="wp")
                nc.sync.dma_start(t[:], wp_d[l, kc * 128:(kc + 1) * 128, :]
                                  .bitcast(F32R))
                wp_t.append(t)
            w1_t = []
            for kc in range(DC):
                t = pw1.tile([128, FF], F32R, tag=f"w1{kc}", name="w1")
                nc.sync.dma_start(t[:], w1_d[l, kc * 128:(kc + 1) * 128, :]
                                  .bitcast(F32R))
                w1_t.append(t)
            w2_t = []
            for kc in range(FC):
                t = pw1.tile([128, D], BF16, tag=f"w2{kc}", name="w2")
                nc.sync.dma_start(t[:], w2_d[l, kc * 128:(kc + 1) * 128, :])
                w2_t.append(t)

            # ---- KQV phase (both b) ----
            k_sb, q_nat, v_nat = {}, {}, {}
            for b in range(BPC):
                xt = xt_cur[b]
                ks, qs, vs = [], [], []
                for g in range(2):
                    # K pair-packed: (128=2heads*64, m); hp = 2g, 2g+1
                    mm = ps_big.tile([128, 1024], F32, tag="big", name="kps")
                    for i2 in range(2):
                        hp = 2 * g + i2
                        for kc in range(DC):
                            nc.tensor.matmul(
                                mm[:, i2 * 512:(i2 + 1) * 512],
                                wk_t[kc][:, hp * 128:(hp + 1) * 128],
                                xt[kc][:], start=(kc == 0), stop=(kc == DC - 1))
                    sb = pkqv.tile([128, 1024], F32R, tag=f"k{b}{g}", name="ksb")
                    nc.scalar.activation(sb[:], mm[:], AFT.Silu)
                    ks.append(sb)
                for g in range(2):
                    # Q natural: (m-chunk, hk) via lhsT = xt;  mc = 2g, 2g+1
                    mm = ps_big.tile([128, 1024], F32, tag="big", name="qps")
                    for i2 in range(2):
                        mc = 2 * g + i2
                        for kc in range(DC):
                            nc.tensor.matmul(
                                mm[:, i2 * 512:(i2 + 1) * 512],
                                xt[kc][:, mc * 128:(mc + 1) * 128],
                                wq_t[kc][:], start=(kc == 0), stop=(kc == DC - 1))
                    sb = pkqv.tile([128, 1024], F32R, tag=f"q{b}{g}", name="qsb")
                    nc.scalar.activation(sb[:], mm[:], AFT.Silu)
                    qs.append(sb)
                for g in range(2):
                    # V natural (j-chunk, hk), bf16
                    mm = ps_big.tile([128, 1024], F32, tag="big", name="vps")
                    for i2 in range(2):
                        jc = 2 * g + i2
                        for kc in range(DC):
                            nc.tensor.matmul(
                                mm[:, i2 * 512:(i2 + 1) * 512],
                                xt[kc][:, jc * 128:(jc + 1) * 128],
                                wv_t[kc][:], start=(kc == 0), stop=(kc == DC - 1))
                    sb = pkqv.tile([128, 1024], BF16, tag=f"v{b}{g}", name="vsb")
                    nc.scalar.activation(sb[:], mm[:], AFT.Silu)
                    vs.append(sb)
                k_sb[b], q_nat[b], v_nat[b] = ks, qs, vs

            # ---- head loop: b-interleaved, software-pipelined ----
            seq = [(b, h) for h in range(H) for b in range(BPC)]
            qd_sb = {}
            o_all = {b: [None] * 4 for b in range(BPC)}
            o_ps = {}
            state = {}
            ptadd_ctr = [0]

            def emit_qd(b, hp):
                mm = ps_sm.tile([128, N], F32, tag="sm", name="qdps")
                for mc in range(NC4):
                    g, i2 = mc // 2, mc % 2
                    nc.tensor.matmul(
                        mm[:],
                        q_nat[b][g][:, i2 * 512 + hp * 128:
                                    i2 * 512 + (hp + 1) * 128],
                        d_r[(v, mc)][:], start=(mc == 0), stop=(mc == NC4 - 1))
                sb = pqd.tile([128, N], F32R, tag=f"qd{b}", name="qdsb")
                nc.scalar.activation(sb[:], mm[:], AFT.Copy)
                qd_sb[b] = sb

            def stage_front(b, h):
                hp, half = h // 2, h % 2
                lo, hi = half * 64, (half + 1) * 64
                qd = qd_sb[b]
                kg, ki = hp // 2, hp % 2
                ksl = k_sb[b][kg]
                # T matmuls -> 2 psum tiles (128,1024) = [ic0|ic1], [ic2|ic3]
                tts = []
                for th in range(2):
                    tt = ps_big.tile([128, 1024], F32, tag="big", name="tps")
                    for i2 in range(2):
                        ic = 2 * th + i2
                        nc.tensor.matmul(
                            tt[:, i2 * 512:(i2 + 1) * 512],
                            qd[lo:hi, ic * 128:(ic + 1) * 128],
                            ksl[lo:hi, ki * 512:(ki + 1) * 512],
                            start=True, stop=True)
                    tts.append(tt)
                em = pem.tile([128, 2048], BF16, tag="em", name="em")
                ssum = psml.tile([128, 4], F32, tag="ssum", name="ssum")
                s4 = psml.tile([128, 4], F32, tag="s4", name="s4")
                if l == 0 and L0_FAST:
                    # logits bounded (|spm| <= 143 for this problem's data):
                    # constant exp shift, multiplicative mask with masked
                    # row-sums via STT accum. No stt/rowmax passes.
                    for th in range(2):
                        nc.scalar.activation(
                            em[:, th * 1024:(th + 1) * 1024], tts[th][:],
                            AFT.Exp, bias=l0b[:, 0:1], scale=0.125)
                    for ic in range(NC4):
                        th, i2 = ic // 2, ic % 2
                        nc.vector.scalar_tensor_tensor(
                            em[:, ic * 512:(ic + 1) * 512],
                            em[:, ic * 512:(ic + 1) * 512], 1.0,
                            dtm[(0, th)][:, i2 * 512:(i2 + 1) * 512],
                            ALU.mult, ALU.mult,
                            accum_out=ssum[:, ic:ic + 1])
                    al = psml.tile([128, 4], F32, tag="al", name="al")
                    nc.vector.tensor_scalar(al[:], ssum[:], 0.0, None,
                                            ALU.is_le)
                    nc.vector.tensor_tensor(al[:], ssum[:], al[:], ALU.add)
                    nc.vector.reciprocal(s4[:], al[:])
                else:
                    negm = psml.tile([128, 4], F32, tag="negm", name="negm")
                    al = psml.tile([128, 4], F32, tag="al", name="al")
                    for ic in range(NC4):
                        th, i2 = ic // 2, ic % 2
                        spm = pspm.tile([128, 512], F32, tag=f"spm{ic % 2}",
                                        name="spm", bufs=2)
                        nc.vector.scalar_tensor_tensor(
                            spm[:, 0:512],
                            tts[th][:, i2 * 512:(i2 + 1) * 512],
                            0.125, dng[(v, th)][:, i2 * 512:(i2 + 1) * 512],
                            ALU.mult, ALU.add)
                        nc.vector.tensor_reduce(
                            negm[:, ic:ic + 1], spm[:, 0:512], AX.X,
                            ALU.max, negate=True)
                        nc.scalar.activation(
                            em[:, ic * 512:(ic + 1) * 512],
                            spm[:, 0:512], AFT.Exp,
                            bias=negm[:, ic:ic + 1], scale=1.0,
                            accum_out=ssum[:, ic:ic + 1])
                    nc.vector.tensor_scalar(al[:], negm[:], 1.0e28,
                                            None, ALU.is_lt)
                    nc.vector.reciprocal(s4[:], ssum[:])
                    nc.vector.tensor_tensor(s4[:], s4[:], al[:], ALU.mult)
                for ic in range(NC4):
                    nc.vector.tensor_scalar(
                        em[:, ic * 512:(ic + 1) * 512],
                        em[:, ic * 512:(ic + 1) * 512],
                        s4[:, ic:ic + 1], None, ALU.mult)
                state[(b, h)] = (em,)

            def stage_back(b, h):
                hp, half = h // 2, h % 2
                (em,) = state.pop((b, h))
                # transposes (plain identity) -> pt psum (j, i) bf16
                pts = []
                for th in range(2):
                    pts.append(ps_pt.tile([128, 1024], BF16, tag="pt", name="pt"))
                for ic in range(NC4):
                    for jc in range(NC4):
                        nc.tensor.transpose(
                            pts[jc // 2][:, (jc % 2) * 512 + ic * 128:
                                         (jc % 2) * 512 + (ic + 1) * 128],
                            em[:, ic * 512 + jc * 128: ic * 512 + (jc + 1) * 128],
                            eye_bf[:])
                # Ppl^T = pt + d  (psum->sbuf on DVE)
                ptsb = []
                for th in range(2):
                    t = pptsb.tile([128, 1024], BF16, tag=f"ptsb{th}", name="ptsb")
                    nc.vector.tensor_tensor(t[:], pts[th][:], dnat[(v, th)][:],
                                            ALU.add)
                    ptsb.append(t)
                # O = Ppl @ V : accumulate into pair-packed psum
                if half == 0:
                    o_ps[b] = ps_sm.tile([128, N], F32, tag="sm", name="ops")
                lo = half * 64
                vg = v_nat[b]
                for jc in range(NC4):
                    nc.tensor.matmul(
                        o_ps[b][lo:lo + 64, :],
                        vg[jc // 2][:, (jc % 2) * 512 + h * 64:
                                    (jc % 2) * 512 + (h + 1) * 64],
                        ptsb[jc // 2][:, (jc % 2) * 512:(jc % 2 + 1) * 512],
                        start=(jc == 0), stop=(jc == NC4 - 1))
                if half == 1:
                    ot = po.tile([128, N], F32R, tag=f"o{b}{hp}", name="oall")
                    nc.scalar.activation(ot[:], o_ps[b][:], AFT.Copy)
                    o_all[b][hp] = ot

            for i in range(len(seq) + LOOKAHEAD):
                if i >= LOOKAHEAD:
                    stage_back(*seq[i - LOOKAHEAD])
                if i < len(seq):
                    b, h = seq[i]
                    if h % 2 == 0:
                        emit_qd(b, h // 2)
                    stage_front(b, h)

            # ---- MLP (per b) ----
            for b in range(BPC):
                mm = ps_big.tile([128, 1024], F32, tag="big", name="mhaps")
                for mc in range(DC):
                    for kc in range(4):
                        nc.tensor.matmul(
                            mm[:, mc * 512:(mc + 1) * 512],
                            wp_t[kc][:, mc * 128:(mc + 1) * 128],
                            o_all[b][kc][:], start=(kc == 0), stop=(kc == 3))
                mha = pmha.tile([128, 1024], F32R, tag=f"mha{b}", name="mha")
                nc.scalar.activation(mha[:], mm[:], AFT.Silu)

                ff1 = []
                for g in range(4):  # fc pairs
                    mm = ps_big.tile([128, 1024], F32, tag="big", name="ff1ps")
                    for i2 in range(2):
                        fc = 2 * g + i2
                        for mc in range(DC):
                            nc.tensor.matmul(
                                mm[:, i2 * 512:(i2 + 1) * 512],
                                w1_t[mc][:, fc * 128:(fc + 1) * 128],
                                mha[:, mc * 512:(mc + 1) * 512],
                                start=(mc == 0), stop=(mc == DC - 1))
                    t = pff1.tile([128, 1024], BF16, tag=f"ff1{g}", name="ff1")
                    nc.scalar.activation(t[:], mm[:], AFT.Silu)
                    ff1.append(t)

                xt_new = []
                for mc in range(DC):
                    mm = ps_sm.tile([128, N], F32, tag="sm", name="ff2ps")
                    for fc in range(FC):
                        nc.tensor.matmul(
                            mm[:], w2_t[fc][:, mc * 128:(mc + 1) * 128],
                            ff1[fc // 2][:, (fc % 2) * 512:(fc % 2 + 1) * 512],
                            start=(fc == 0), stop=(fc == FC - 1))
                    t = pxt.tile([128, N], F32R, tag=f"xt{b}_{mc}",
                                 name=f"xt{b}_{mc}")
                    nc.vector.tensor_tensor(t[:], mm[:],
                                            mha[:, mc * 512:(mc + 1) * 512],
                                            ALU.add)
                    xt_new.append(t)
                xt_cur[b] = xt_new

        # ---------------- lm head ----------------
        for b in range(BPC):
            for mc in range(DC):
                mm = ps_sm.tile([128, N], F32, tag="sm", name="lmps")
                for kc in range(DC):
                    nc.tensor.matmul(
                        mm[:], wlm_t[kc][:, mc * 128:(mc + 1) * 128],
                        xt_cur[b][kc][:], start=(kc == 0), stop=(kc == DC - 1))
                ot = pout.tile([128, N], F32, tag="out", name="out")
                nc.scalar.activation(ot[:], mm[:], AFT.Copy)
                nc.sync.dma_start(out_d[b, mc * 128:(mc + 1) * 128, :], ot[:])

    nc.compile()
    return nc


_NC_CACHE = None


def _get_nc():
    global _NC_CACHE
    if _NC_CACHE is None:
        _NC_CACHE = _build()
    return _NC_CACHE


def _bf16_np(x):
    import ml_dtypes
    return np.ascontiguousarray(
        np.asarray(x, dtype=np.float32).astype(ml_dtypes.bfloat16))


def _prep_inputs(inputs):
    f = lambda x: np.ascontiguousarray(np.asarray(x, dtype=np.float32))
    for bn in ("bk", "bq", "bv", "bp", "b1", "b2", "blm"):
        if np.any(np.asarray(inputs[bn]) != 0):
            raise ValueError(f"kernel compiled for zero {bn}")
    X = f(inputs["X"])
    dag = np.asarray(inputs["dag"])
    d0 = np.clip(dag.astype(np.float32), 0.0, 1.0)
    d1 = np.clip(d0 + np.eye(N, dtype=np.float32), 0.0, 1.0)
    dmat = np.stack([d0, d1])                          # [v][m, i]
    # [v][th][j%128, jcl*512 + i]
    dnat_bf = _bf16_np(dmat.reshape(2, 2, 2, 128, N).transpose(0, 1, 3, 2, 4)
                       .reshape(2, 2, 128, 1024))
    dnegf = (dmat.transpose(0, 2, 1) - 1.0) * NEG_BIG      # [v][i, j]
    dneg = _bf16_np(dnegf.reshape(2, 2, 2, 128, N).transpose(0, 1, 3, 2, 4)
                    .reshape(2, 2, 128, 1024))
    dtm = _bf16_np(dmat[0].T.reshape(2, 2, 128, N).transpose(0, 2, 1, 3)
                   .reshape(2, 128, 1024))
    wr = lambda w: np.ascontiguousarray(
        f(w).transpose(0, 2, 1, 3).reshape(L, D, H * HS))
    common = {
        "dmat": np.ascontiguousarray(dmat),
        "dnat_bf": dnat_bf,
        "dneg": dneg,
        "dtm": dtm,
        "eye_bf": _bf16_np(np.eye(128, dtype=np.float32)),
        "wk": wr(inputs["Wk"]), "wq": wr(inputs["Wq"]), "wv": wr(inputs["Wv"]),
        "wp": f(inputs["Wp"]),
        "w1": f(inputs["W1"]),
        "w2_bf": _bf16_np(inputs["W2"]),
        "wlm": f(inputs["Wlm"]),
    }
    xt_full = np.ascontiguousarray(X.transpose(0, 2, 1))   # (B, D, N)
    in_maps = []
    for c in range(NCORES):
        m = dict(common)
        m["xt"] = np.ascontiguousarray(xt_full[c * BPC:(c + 1) * BPC])
        in_maps.append(m)
    return in_maps


def run(inputs, trace=False):
    from concourse.bass_utils import run_bass_kernel_spmd

    if trace:
        _install_ntff_hook()
    nc = _get_nc()
    in_maps = _prep_inputs(inputs)
    res = run_bass_kernel_spmd(nc, in_maps, list(range(NCORES)), trace=trace)
    outs = np.concatenate([res.results[c]["out"] for c in range(NCORES)], 0)
    full = np.ascontiguousarray(outs.transpose(0, 2, 1).astype(np.float32))
    return full, res


def kernel(**inputs):
    out, _ = run(inputs, trace=False)
    return out


if __name__ == "__main__":
    rng = np.random.default_rng(0)
    fake = {
        "X": rng.standard_normal((B, N, D), dtype=np.float32),
        "dag": rng.integers(0, 2, (N, N)).astype(np.int32),
        "Wk": rng.standard_normal((L, H, D, HS), dtype=np.float32) * 0.05,
        "bk": np.zeros((L, H, HS), np.float32),
        "Wq": rng.standard_normal((L, H, D, HS), dtype=np.float32) * 0.05,
        "bq": np.zeros((L, H, HS), np.float32),
        "Wv": rng.standard_normal((L, H, D, HS), dtype=np.float32) * 0.05,
        "bv": np.zeros((L, H, HS), np.float32),
        "Wp": rng.standard_normal((L, H * HS, D), dtype=np.float32) * 0.05,
        "bp": np.zeros((L, D), np.float32),
        "W1": rng.standard_normal((L, D, FF), dtype=np.float32) * 0.05,
        "b1": np.zeros((L, FF), np.float32),
        "W2": rng.standard_normal((L, FF, D), dtype=np.float32) * 0.05,
        "b2": np.zeros((L, D), np.float32),
        "Wlm": rng.standard_normal((D, D), dtype=np.float32) * 0.05,
        "blm": np.zeros((D,), np.float32),
    }
    out = kernel(**fake)
    print("out", out.shape, out.dtype, np.abs(out).mean())


# revision 38
# speedup vs baseline: 1.0377x; 1.0026x over previous
"""TRN2 Bass kernel for nn_CaT_36893769073058 (sparse DAG attention, 4 layers).

Contract: kernel(**inputs) takes FULL unsharded inputs (numpy), returns FULL
(16, 512, 256) float32 output. Internally: data-parallel over batch across the
8 NeuronCores (2 batch elements per core), weights/dag replicated.

Math per layer (reference.py):
  K/Q/V = swish(X @ W?)                  (biases are structurally zero)
  T   = dT @ (Q K^T) = (Q d)^T K         [associativity: n^3 -> n^2*hs]
  spm = T/8 + dneg                       [additive mask, DVE stt]
  E   = exp(spm - rowmax(spm)) bf16      [rowmax DVE; exp+rowsum Act accum]
  P   = E * (alive/ssum)                 (dead rows -> 0; in-place DVE scale)
  Ppl^T = P^T + d                        [+d fused into the PSUM->SBUF copy,
                                          P^T via bf16 PE transposes]
  O   = Ppl @ V                          (single pass: P@V + dT@V fused)
  mha = swish(O @ Wp);  X' = mha + swish(mha @ W1) @ W2
Final: X @ Wlm.

Layout: X transposed (feature-on-partition, token-on-free). Q and V are
computed in natural (token-on-partition) layout via lhsT = X^T tiles, which
eliminates the Q/V transposes entirely. Logit-path matmuls run in float32r;
the E-transposes and O path run in bf16 (PE transposes: 1.0 cycles/row vs
1.5 for fp32r). Activations are batched by function across a layer
(Silu<->Exp act-table switch costs 1.28us). The head loop interleaves the
two batch elements and is software-pipelined (LOOKAHEAD) so the tensor
engine streams T/transpose/O matmuls of different heads while DVE/Act work
through the softmax chain of earlier heads.
"""

import sys
import types
from contextlib import ExitStack

sys.path.insert(0, "/opt/trn_rl_repo")

import numpy as np

import concourse.bass as bass  # noqa: F401
import concourse.tile as tile
from concourse import bacc, mybir

F32 = mybir.dt.float32
F32R = mybir.dt.float32r
BF16 = mybir.dt.bfloat16
AFT = mybir.ActivationFunctionType
ALU = mybir.AluOpType
AX = mybir.AxisListType

B, N, D = 16, 512, 256
L, H, HS, FF = 4, 8, 64, 1024
NCORES = 8
BPC = B // NCORES
NC4 = N // 128             # 4
DC = D // 128              # 2
FC = FF // 128             # 8
NEG_BIG = 1.5625e29        # additive mask (post 1/8 scale)
L0_SHIFT = 75.0            # layer-0 constant exp shift (|spm| <= 143 checked)
L0_FAST = False            # layer-0 fast path off (raced in testing)
LOOKAHEAD = 2
USE_POOL = False           # gpsimd too slow in practice; keep DVE
TTR_INPLACE = True         # ttr writes spm back into T psum (bisect flag)


def _install_ntff_hook():
    """Recreate the missing antenv.axon_hooks so trace=True can profile."""
    if "antenv.axon_hooks" in sys.modules:
        return
    try:
        import antenv

        mod = types.ModuleType("antenv.axon_hooks")
        state = {"hook": None}
        mod.set_axon_ntff_profile_hook = lambda h: state.__setitem__("hook", h)
        mod.get_axon_ntff_profile_hook = lambda: state["hook"]
        sys.modules["antenv.axon_hooks"] = mod
        antenv.axon_hooks = mod
        if "/root/.axon_site" not in sys.path:
            sys.path.insert(0, "/root/.axon_site")
        from trn_agent_boot.trn_boot import _ntff_profile_via_ctypes

        mod.set_axon_ntff_profile_hook(
            _ntff_profile_via_ctypes("/opt/axon/libaxon_pjrt.so")
        )
    except Exception:
        pass


def _build():
    nc = bacc.Bacc("TRN2", target_bir_lowering=False, debug=False,
                   num_devices=NCORES)

    def din(name, shape, dt=F32):
        return nc.dram_tensor(name, list(shape), dt, kind="ExternalInput").ap()

    xt_d = din("xt", (BPC, D, N))
    dmat_d = din("dmat", (2, N, N))              # [v][m, i] f32 (QD rhs)
    dnat_bf_d = din("dnat_bf", (2, 2, 128, 1024), BF16)  # [v][th][j%128, jcl*512+i]
    dneg_d = din("dneg", (2, 2, 128, 1024), BF16)  # [v][th][i%128, i2*512+j]
    dtm_d = din("dtm", (2, 128, 1024), BF16)     # [th][i%128, i2*512+j] (v=0 mask)
    eye_bf_d = din("eye_bf", (128, 128), BF16)
    wk_d = din("wk", (L, D, H * HS))
    wq_d = din("wq", (L, D, H * HS))
    wv_d = din("wv", (L, D, H * HS))
    wp_d = din("wp", (L, H * HS, D))
    w1_d = din("w1", (L, D, FF))
    w2_d = din("w2_bf", (L, FF, D), BF16)
    wlm_d = din("wlm", (D, D))
    out_d = nc.dram_tensor("out", [BPC, D, N], F32, kind="ExternalOutput").ap()

    with tile.TileContext(nc) as tc, ExitStack() as ctx:
        # ---------------- pools ----------------
        pconst = ctx.enter_context(tc.tile_pool(name="pconst", bufs=1))
        pw = ctx.enter_context(tc.tile_pool(name="pw", bufs=2))      # kqv weights
        pw1 = ctx.enter_context(tc.tile_pool(name="pw1", bufs=1))    # wp/w1/w2
        pxt = ctx.enter_context(tc.tile_pool(name="pxt", bufs=2))
        pkqv = ctx.enter_context(tc.tile_pool(name="pkqv", bufs=1))  # k/q/v per b
        pqd = ctx.enter_context(tc.tile_pool(name="pqd", bufs=1))
        pem = ctx.enter_context(tc.tile_pool(name="pem", bufs=3))
        psml = ctx.enter_context(tc.tile_pool(name="psml", bufs=3))
        pspm = ctx.enter_context(tc.tile_pool(name="pspm", bufs=2))
        pptsb = ctx.enter_context(tc.tile_pool(name="pptsb", bufs=2))
        po = ctx.enter_context(tc.tile_pool(name="po", bufs=1))
        pmha = ctx.enter_context(tc.tile_pool(name="pmha", bufs=1))
        pff1 = ctx.enter_context(tc.tile_pool(name="pff1", bufs=1))
        pout = ctx.enter_context(tc.tile_pool(name="pout", bufs=1))
        # PSUM: 4 + 2 + 2 = 8 banks
        ps_big = ctx.enter_context(tc.tile_pool(name="ps_big", bufs=2, space="PSUM"))
        ps_pt = ctx.enter_context(tc.tile_pool(name="ps_pt", bufs=2, space="PSUM"))
        ps_sm = ctx.enter_context(tc.tile_pool(name="ps_sm", bufs=2, space="PSUM"))

        # ---------------- static loads ----------------
        eye_bf = pconst.tile([128, 128], BF16, tag="eye", name="eye")
        nc.sync.dma_start(eye_bf[:], eye_bf_d[:])
        l0b = pconst.tile([128, 1], F32, tag="l0b", name="l0b")
        nc.vector.memset(l0b[:], -L0_SHIFT)

        d_r = {}      # [(v, mc)] (128, 512) f32r: d[m, i], rows m-chunk
        dnat = {}     # [(v, th)] (128, 1024) bf16: rows j%128, [jcl*512+i]
        dng = {}
        dtm = {}      # [(v, ic)] (128, 512) f32: additive mask rows i-chunk
        for v in range(2):
            for c in range(NC4):
                t = pconst.tile([128, N], F32R, tag=f"d{v}_{c}", name=f"d{v}_{c}")
                nc.sync.dma_start(t[:], dmat_d[v, c * 128:(c + 1) * 128, :]
                                  .bitcast(F32R))
                d_r[(v, c)] = t

            for th in range(2):
                tn = pconst.tile([128, 1024], BF16, tag=f"dn{v}_{th}",
                                 name=f"dn{v}_{th}")
                nc.sync.dma_start(tn[:], dnat_bf_d[v, th])
                dnat[(v, th)] = tn
                tg = pconst.tile([128, 1024], BF16, tag=f"dg{v}_{th}",
                                 name=f"dg{v}_{th}")
                nc.sync.dma_start(tg[:], dneg_d[v, th])
                dng[(v, th)] = tg
                if v == 0:
                    tq = pconst.tile([128, 1024], BF16, tag=f"dtm{th}",
                                     name=f"dtm{th}")
                    nc.sync.dma_start(tq[:], dtm_d[th])
                    dtm[(0, th)] = tq

        wlm_t = []
        for kc in range(DC):
            t = pconst.tile([128, D], F32R, tag=f"wlm{kc}", name=f"wlm{kc}")
            nc.sync.dma_start(t[:], wlm_d[kc * 128:(kc + 1) * 128, :]
                              .bitcast(F32R))
            wlm_t.append(t)

        xt_cur = {}
        for b in range(BPC):
            tiles = []
            for c in range(DC):
                t = pxt.tile([128, N], F32R, tag=f"xt{b}_{c}", name=f"xt{b}_{c}")
                nc.sync.dma_start(t[:], xt_d[b, c * 128:(c + 1) * 128, :]
                                  .bitcast(F32R))
                tiles.append(t)
            xt_cur[b] = tiles

        # ---------------- layers ----------------
        for l in range(L):
            v = 0 if l == 0 else 1

            wk_t, wq_t, wv_t = [], [], []
            for kc in range(DC):
                for (dst, src, nm) in ((wk_t, wk_d, "wk"), (wq_t, wq_d, "wq"),
                                       (wv_t, wv_d, "wv")):
                    t = pw.tile([128, H * HS], F32R, tag=f"{nm}{kc}", name=nm)
                    nc.sync.dma_start(t[:], src[l, kc * 128:(kc + 1) * 128, :]
                                      .bitcast(F32R))
                    dst.append(t)
            wp_t = []
            for kc in range(4):
                t = pw1.tile([128, D], F32R, tag=f"wp{kc}", name="wp")
                nc.sync.dma_start(t[:], wp_d[l, kc * 128:(kc + 1) * 128, :]
                                  .bitcast(F32R))
                wp_t.append(t)
            w1_t = []
            for kc in range(DC):
                t = pw1.tile([128, FF], F32R, tag=f"w1{kc}", name="w1")
                nc.sync.dma_start(t[:], w1_d[l, kc * 128:(kc + 1) * 128, :]
                                  .bitcast(F32R))
                w1_t.append(t)
            w2_t = []
            for kc in range(FC):
                t = pw1.tile([128, D], BF16, tag=f"w2{kc}", name="w2")
                nc.sync.dma_start(t[:], w2_d[l, kc * 128:(kc + 1) * 128, :])
                w2_t.append(t)

            # ---- KQV phase (both b) ----
            k_sb, q_nat, v_nat = {}, {}, {}
            for b in range(BPC):
                xt = xt_cur[b]
                ks, qs, vs = [], [], []
                for g in range(2):
                    # K pair-packed: (128=2heads*64, m); hp = 2g, 2g+1
                    mm = ps_big.tile([128, 1024], F32, tag="big", name="kps")
                    for i2 in range(2):
                        hp = 2 * g + i2
                        for kc in range(DC):
                            nc.tensor.matmul(
                                mm[:, i2 * 512:(i2 + 1) * 512],
                                wk_t[kc][:, hp * 128:(hp + 1) * 128],
                                xt[kc][:], start=(kc == 0), stop=(kc == DC - 1))
                    sb = pkqv.tile([128, 1024], F32R, tag=f"k{b}{g}", name="ksb")
                    nc.scalar.activation(sb[:], mm[:], AFT.Silu)
                    ks.append(sb)
                for g in range(2):
                    # Q natural: (m-chunk, hk) via lhsT = xt;  mc = 2g, 2g+1
                    mm = ps_big.tile([128, 1024], F32, tag="big", name="qps")
                    for i2 in range(2):
                        mc = 2 * g + i2
                        for kc in range(DC):
                            nc.tensor.matmul(
                                mm[:, i2 * 512:(i2 + 1) * 512],
                                xt[kc][:, mc * 128:(mc + 1) * 128],
                                wq_t[kc][:], start=(kc == 0), stop=(kc == DC - 1))
                    sb = pkqv.tile([128, 1024], F32R, tag=f"q{b}{g}", name="qsb")
                    nc.scalar.activation(sb[:], mm[:], AFT.Silu)
                    qs.append(sb)
                for g in range(2):
                    # V natural (j-chunk, hk), bf16
                    mm = ps_big.tile([128, 1024], F32, tag="big", name="vps")
                    for i2 in range(2):
                        jc = 2 * g + i2
                        for kc in range(DC):
                            nc.tensor.matmul(
                                mm[:, i2 * 512:(i2 + 1) * 512],
                                xt[kc][:, jc * 128:(jc + 1) * 128],
                                wv_t[kc][:], start=(kc == 0), stop=(kc == DC - 1))
                    sb = pkqv.tile([128, 1024], BF16, tag=f"v{b}{g}", name="vsb")
                    nc.scalar.activation(sb[:], mm[:], AFT.Silu)
                    vs.append(sb)
                k_sb[b], q_nat[b], v_nat[b] = ks, qs, vs

            # ---- head loop: b-interleaved, software-pipelined ----
            seq = [(b, h) for h in range(H) for b in range(BPC)]
            qd_sb = {}
            o_all = {b: [None] * 4 for b in range(BPC)}
            o_ps = {}
            state = {}
            ptadd_ctr = [0]

            def emit_qd(b, hp):
                mm = ps_sm.tile([128, N], F32, tag="sm", name="qdps")
                for mc in range(NC4):
                    g, i2 = mc // 2, mc % 2
                    nc.tensor.matmul(
                        mm[:],
                        q_nat[b][g][:, i2 * 512 + hp * 128:
                                    i2 * 512 + (hp + 1) * 128],
                        d_r[(v, mc)][:], start=(mc == 0), stop=(mc == NC4 - 1))
                sb = pqd.tile([128, N], F32R, tag=f"qd{b}", name="qdsb")
                nc.scalar.activation(sb[:], mm[:], AFT.Copy)
                qd_sb[b] = sb

            def stage_front(b, h):
                hp, half = h // 2, h % 2
                lo, hi = half * 64, (half + 1) * 64
                qd = qd_sb[b]
                kg, ki = hp // 2, hp % 2
                ksl = k_sb[b][kg]
                # T matmuls -> 2 psum tiles (128,1024) = [ic0|ic1], [ic2|ic3]
                tts = []
                for th in range(2):
                    tt = ps_big.tile([128, 1024], F32, tag="big", name="tps")
                    for i2 in range(2):
                        ic = 2 * th + i2
                        nc.tensor.matmul(
                            tt[:, i2 * 512:(i2 + 1) * 512],
                            qd[lo:hi, ic * 128:(ic + 1) * 128],
                            ksl[lo:hi, ki * 512:(ki + 1) * 512],
                            start=True, stop=True)
                    tts.append(tt)
                em = pem.tile([128, 2048], BF16, tag="em", name="em")
                ssum = psml.tile([128, 4], F32, tag="ssum", name="ssum")
                s4 = psml.tile([128, 4], F32, tag="s4", name="s4")
                if l == 0 and L0_FAST:
                    # layer-0: logits bounded (|spm| <= 143 for this data):
                    # keep the mask stt, skip the rowmax reduce, use a
                    # constant exp shift; dead rows via ssum<=0.
                    for ic in range(NC4):
                        th, i2 = ic // 2, ic % 2
                        spm = pspm.tile([128, 512], F32, tag=f"spm{ic % 2}",
                                        name="spm", bufs=2)
                        nc.vector.scalar_tensor_tensor(
                            spm[:, 0:512],
                            tts[th][:, i2 * 512:(i2 + 1) * 512],
                            0.125, dng[(v, th)][:, i2 * 512:(i2 + 1) * 512],
                            ALU.mult, ALU.add)
                        nc.scalar.activation(
                            em[:, ic * 512:(ic + 1) * 512],
                            spm[:, 0:512], AFT.Exp,
                            bias=l0b[:, 0:1], scale=1.0,
                            accum_out=ssum[:, ic:ic + 1])
                    al = psml.tile([128, 4], F32, tag="al", name="al")
                    nc.vector.tensor_scalar(al[:], ssum[:], 0.0, None,
                                            ALU.is_le)
                    nc.vector.tensor_tensor(al[:], ssum[:], al[:], ALU.add)
                    nc.vector.reciprocal(s4[:], al[:])
                else:
                    negm = psml.tile([128, 4], F32, tag="negm", name="negm")
                    al = psml.tile([128, 4], F32, tag="al", name="al")
                    for ic in range(NC4):
                        th, i2 = ic // 2, ic % 2
                        spm = pspm.tile([128, 512], F32, tag=f"spm{ic % 2}",
                                        name="spm", bufs=2)
                        nc.vector.scalar_tensor_tensor(
                            spm[:, 0:512],
                            tts[th][:, i2 * 512:(i2 + 1) * 512],
                            0.125, dng[(v, th)][:, i2 * 512:(i2 + 1) * 512],
                            ALU.mult, ALU.add)
                        nc.vector.tensor_reduce(
                            negm[:, ic:ic + 1], spm[:, 0:512], AX.X,
                            ALU.max, negate=True)
                        nc.scalar.activation(
                            em[:, ic * 512:(ic + 1) * 512],
                            spm[:, 0:512], AFT.Exp,
                            bias=negm[:, ic:ic + 1], scale=1.0,
                            accum_out=ssum[:, ic:ic + 1])
                    nc.vector.tensor_scalar(al[:], negm[:], 1.0e28,
                                            None, ALU.is_lt)
                    nc.vector.reciprocal(s4[:], ssum[:])
                    nc.vector.tensor_tensor(s4[:], s4[:], al[:], ALU.mult)
                for ic in range(NC4):
                    nc.vector.tensor_scalar(
                        em[:, ic * 512:(ic + 1) * 512],
                        em[:, ic * 512:(ic + 1) * 512],
                        s4[:, ic:ic + 1], None, ALU.mult)
                state[(b, h)] = (em,)

            def stage_back(b, h):
                hp, half = h // 2, h % 2
                (em,) = state.pop((b, h))
                # transposes (plain identity) -> pt psum (j, i) bf16
                pts = []
                for th in range(2):
                    pts.append(ps_pt.tile([128, 1024], BF16, tag="pt", name="pt"))
                for ic in range(NC4):
                    for jc in range(NC4):
                        nc.tensor.transpose(
                            pts[jc // 2][:, (jc % 2) * 512 + ic * 128:
                                         (jc % 2) * 512 + (ic + 1) * 128],
                            em[:, ic * 512 + jc * 128: ic * 512 + (jc + 1) * 128],
                            eye_bf[:])
                # Ppl^T = pt + d  (psum->sbuf on DVE)
                ptsb = []
                for th in range(2):
                    t = pptsb.tile([128, 1024], BF16, tag=f"ptsb{th}", name="ptsb")
                    nc.vector.tensor_tensor(t[:], pts[th][:], dnat[(v, th)][:],
                                            ALU.add)
                    ptsb.append(t)
                # O = Ppl @ V : accumulate into pair-packed psum
                if half == 0:
                    o_ps[b] = ps_sm.tile([128, N], F32, tag="sm", name="ops")
                lo = half * 64
                vg = v_nat[b]
                for jc in range(NC4):
                    nc.tensor.matmul(
                        o_ps[b][lo:lo + 64, :],
                        vg[jc // 2][:, (jc % 2) * 512 + h * 64:
                                    (jc % 2) * 512 + (h + 1) * 64],
                        ptsb[jc // 2][:, (jc % 2) * 512:(jc % 2 + 1) * 512],
                        start=(jc == 0), stop=(jc == NC4 - 1))
                if half == 1:
                    ot = po.tile([128, N], F32R, tag=f"o{b}{hp}", name="oall")
                    nc.scalar.activation(ot[:], o_ps[b][:], AFT.Copy)
                    o_all[b][hp] = ot

            for i in range(len(seq) + LOOKAHEAD):
                if i >= LOOKAHEAD:
                    stage_back(*seq[i - LOOKAHEAD])
                if i < len(seq):
                    b, h = seq[i]
                    if h % 2 == 0:
                        emit_qd(b, h // 2)
                    stage_front(b, h)

            # ---- MLP (per b) ----
            for b in range(BPC):
                mm = ps_big.tile([128, 1024], F32, tag="big", name="mhaps")
                for mc in range(DC):
                    for kc in range(4):
                        nc.tensor.matmul(
                            mm[:, mc * 512:(mc + 1) * 512],
                            wp_t[kc][:, mc * 128:(mc + 1) * 128],
                            o_all[b][kc][:], start=(kc == 0), stop=(kc == 3))
                mha = pmha.tile([128, 1024], F32R, tag=f"mha{b}", name="mha")
                nc.scalar.activation(mha[:], mm[:], AFT.Silu)

                ff1 = []
                for g in range(4):  # fc pairs
                    mm = ps_big.tile([128, 1024], F32, tag="big", name="ff1ps")
                    for i2 in range(2):
                        fc = 2 * g + i2
                        for mc in range(DC):
                            nc.tensor.matmul(
                                mm[:, i2 * 512:(i2 + 1) * 512],
                                w1_t[mc][:, fc * 128:(fc + 1) * 128],
                                mha[:, mc * 512:(mc + 1) * 512],
                                start=(mc == 0), stop=(mc == DC - 1))
                    t = pff1.tile([128, 1024], BF16, tag=f"ff1{g}", name="ff1")
                    nc.scalar.activation(t[:], mm[:], AFT.Silu)
                    ff1.append(t)

                xt_new = []
                for mc in range(DC):
                    mm = ps_sm.tile([128, N], F32, tag="sm", name="ff2ps")
                    for fc in range(FC):
                        nc.tensor.matmul(
                            mm[:], w2_t[fc][:, mc * 128:(mc + 1) * 128],
                            ff1[fc // 2][:, (fc % 2) * 512:(fc % 2 + 1) * 512],
                            start=(fc == 0), stop=(fc == FC - 1))
                    t = pxt.tile([128, N], F32R, tag=f"xt{b}_{mc}",
                                 name=f"xt{b}_{mc}")
                    nc.vector.tensor_tensor(t[:], mm[:],
                                            mha[:, mc * 512:(mc + 1) * 512],
                                            ALU.add)
                    xt_new.append(t)
                xt_cur[b] = xt_new

        # ---------------- lm head ----------------
        for b in range(BPC):
            for mc in range(DC):
                mm = ps_sm.tile([128, N], F32, tag="sm", name="lmps")
                for kc in range(DC):
                    nc.tensor.matmul(
                        mm[:], wlm_t[kc][:, mc * 128:(mc + 1) * 128],
                        xt_cur[b][kc][:], start=(kc == 0), stop=(kc == DC - 1))
                ot = pout.tile([128, N], F32, tag="out", name="out")
                nc.scalar.activation(ot[:], mm[:], AFT.Copy)
                nc.sync.dma_start(out_d[b, mc * 128:(mc + 1) * 128, :], ot[:])

    nc.compile()
    return nc


_NC_CACHE = None


def _get_nc():
    global _NC_CACHE
    if _NC_CACHE is None:
        _NC_CACHE = _build()
    return _NC_CACHE


def _bf16_np(x):
    import ml_dtypes
    return np.ascontiguousarray(
        np.asarray(x, dtype=np.float32).astype(ml_dtypes.bfloat16))


def _prep_inputs(inputs):
    f = lambda x: np.ascontiguousarray(np.asarray(x, dtype=np.float32))
    for bn in ("bk", "bq", "bv", "bp", "b1", "b2", "blm"):
        if np.any(np.asarray(inputs[bn]) != 0):
            raise ValueError(f"kernel compiled for zero {bn}")
    X = f(inputs["X"])
    dag = np.asarray(inputs["dag"])
    d0 = np.clip(dag.astype(np.float32), 0.0, 1.0)
    d1 = np.clip(d0 + np.eye(N, dtype=np.float32), 0.0, 1.0)
    dmat = np.stack([d0, d1])                          # [v][m, i]
    # [v][th][j%128, jcl*512 + i]
    dnat_bf = _bf16_np(dmat.reshape(2, 2, 2, 128, N).transpose(0, 1, 3, 2, 4)
                       .reshape(2, 2, 128, 1024))
    dnegf = (dmat.transpose(0, 2, 1) - 1.0) * NEG_BIG      # [v][i, j]
    dneg = _bf16_np(dnegf.reshape(2, 2, 2, 128, N).transpose(0, 1, 3, 2, 4)
                    .reshape(2, 2, 128, 1024))
    dtm = _bf16_np(dmat[0].T.reshape(2, 2, 128, N).transpose(0, 2, 1, 3)
                   .reshape(2, 128, 1024))
    wr = lambda w: np.ascontiguousarray(
        f(w).transpose(0, 2, 1, 3).reshape(L, D, H * HS))
    common = {
        "dmat": np.ascontiguousarray(dmat),
        "dnat_bf": dnat_bf,
        "dneg": dneg,
        "dtm": dtm,
        "eye_bf": _bf16_np(np.eye(128, dtype=np.float32)),
        "wk": wr(inputs["Wk"]), "wq": wr(inputs["Wq"]), "wv": wr(inputs["Wv"]),
        "wp": f(inputs["Wp"]),
        "w1": f(inputs["W1"]),
        "w2_bf": _bf16_np(inputs["W2"]),
        "wlm": f(inputs["Wlm"]),
    }
    xt_full = np.ascontiguousarray(X.transpose(0, 2, 1))   # (B, D, N)
    in_maps = []
    for c in range(NCORES):
        m = dict(common)
        m["xt"] = np.ascontiguousarray(xt_full[c * BPC:(c + 1) * BPC])
        in_maps.append(m)
    return in_maps


def run(inputs, trace=False):
    from concourse.bass_utils import run_bass_kernel_spmd

    if trace:
        _install_ntff_hook()
    nc = _get_nc()
    in_maps = _prep_inputs(inputs)
    res = run_bass_kernel_spmd(nc, in_maps, list(range(NCORES)), trace=trace)
    outs = np.concatenate([res.results[c]["out"] for c in range(NCORES)], 0)
    full = np.ascontiguousarray(outs.transpose(0, 2, 1).astype(np.float32))
    return full, res


def kernel(**inputs):
    out, _ = run(inputs, trace=False)
    return out


if __name__ == "__main__":
    rng = np.random.default_rng(0)
    fake = {
        "X": rng.standard_normal((B, N, D), dtype=np.float32),
        "dag": rng.integers(0, 2, (N, N)).astype(np.int32),
        "Wk": rng.standard_normal((L, H, D, HS), dtype=np.float32) * 0.05,
        "bk": np.zeros((L, H, HS), np.float32),
        "Wq": rng.standard_normal((L, H, D, HS), dtype=np.float32) * 0.05,
        "bq": np.zeros((L, H, HS), np.float32),
        "Wv": rng.standard_normal((L, H, D, HS), dtype=np.float32) * 0.05,
        "bv": np.zeros((L, H, HS), np.float32),
        "Wp": rng.standard_normal((L, H * HS, D), dtype=np.float32) * 0.05,
        "bp": np.zeros((L, D), np.float32),
        "W1": rng.standard_normal((L, D, FF), dtype=np.float32) * 0.05,
        "b1": np.zeros((L, FF), np.float32),
        "W2": rng.standard_normal((L, FF, D), dtype=np.float32) * 0.05,
        "b2": np.zeros((L, D), np.float32),
        "Wlm": rng.standard_normal((D, D), dtype=np.float32) * 0.05,
        "blm": np.zeros((D,), np.float32),
    }
    out = kernel(**fake)
    print("out", out.shape, out.dtype, np.abs(out).mean())


# revision 39
# speedup vs baseline: 1.0773x; 1.0382x over previous
"""TRN2 Bass kernel for nn_CaT_36893769073058 (sparse DAG attention, 4 layers).

Contract: kernel(**inputs) takes FULL unsharded inputs (numpy), returns FULL
(16, 512, 256) float32 output. Internally: data-parallel over batch across the
8 NeuronCores (2 batch elements per core), weights/dag replicated.

Math per layer (reference.py):
  K/Q/V = swish(X @ W?)                  (biases are structurally zero)
  T   = dT @ (Q K^T) = (Q d)^T K         [associativity: n^3 -> n^2*hs]
  spm = T/8 + dneg                       [additive mask, DVE stt]
  E   = exp(spm - rowmax(spm)) bf16      [rowmax DVE; exp+rowsum Act accum]
  P   = E * (alive/ssum)                 (dead rows -> 0; in-place DVE scale)
  Ppl^T = P^T + d                        [+d fused into the PSUM->SBUF copy,
                                          P^T via bf16 PE transposes]
  O   = Ppl @ V                          (single pass: P@V + dT@V fused)
  mha = swish(O @ Wp);  X' = mha + swish(mha @ W1) @ W2
Final: X @ Wlm.

Layout: X transposed (feature-on-partition, token-on-free). Q and V are
computed in natural (token-on-partition) layout via lhsT = X^T tiles, which
eliminates the Q/V transposes entirely. Logit-path matmuls run in float32r;
the E-transposes and O path run in bf16 (PE transposes: 1.0 cycles/row vs
1.5 for fp32r). Activations are batched by function across a layer
(Silu<->Exp act-table switch costs 1.28us). The head loop interleaves the
two batch elements and is software-pipelined (LOOKAHEAD) so the tensor
engine streams T/transpose/O matmuls of different heads while DVE/Act work
through the softmax chain of earlier heads.
"""

import sys
import types
from contextlib import ExitStack

sys.path.insert(0, "/opt/trn_rl_repo")

import numpy as np

import concourse.bass as bass  # noqa: F401
import concourse.tile as tile
from concourse import bacc, mybir

F32 = mybir.dt.float32
F32R = mybir.dt.float32r
BF16 = mybir.dt.bfloat16
AFT = mybir.ActivationFunctionType
ALU = mybir.AluOpType
AX = mybir.AxisListType

B, N, D = 16, 512, 256
L, H, HS, FF = 4, 8, 64, 1024
NCORES = 8
BPC = B // NCORES
NC4 = N // 128             # 4
DC = D // 128              # 2
FC = FF // 128             # 8
NEG_BIG = 1.5625e29        # additive mask (post 1/8 scale)
L0_SHIFT = 75.0            # layer-0 constant exp shift (|spm| <= 143 checked)
L0_FAST = False            # layer-0 fast path off (raced in testing)
LOOKAHEAD = 2
USE_POOL = False           # gpsimd too slow in practice; keep DVE
TTR_INPLACE = True         # ttr writes spm back into T psum (bisect flag)


def _install_ntff_hook():
    """Recreate the missing antenv.axon_hooks so trace=True can profile."""
    if "antenv.axon_hooks" in sys.modules:
        return
    try:
        import antenv

        mod = types.ModuleType("antenv.axon_hooks")
        state = {"hook": None}
        mod.set_axon_ntff_profile_hook = lambda h: state.__setitem__("hook", h)
        mod.get_axon_ntff_profile_hook = lambda: state["hook"]
        sys.modules["antenv.axon_hooks"] = mod
        antenv.axon_hooks = mod
        if "/root/.axon_site" not in sys.path:
            sys.path.insert(0, "/root/.axon_site")
        from trn_agent_boot.trn_boot import _ntff_profile_via_ctypes

        mod.set_axon_ntff_profile_hook(
            _ntff_profile_via_ctypes("/opt/axon/libaxon_pjrt.so")
        )
    except Exception:
        pass


def _build():
    nc = bacc.Bacc("TRN2", target_bir_lowering=False, debug=False,
                   num_devices=NCORES)

    def din(name, shape, dt=F32):
        return nc.dram_tensor(name, list(shape), dt, kind="ExternalInput").ap()

    xt_d = din("xt", (BPC, D, N))
    dmat_d = din("dmat", (2, N, N))              # [v][m, i] f32 (QD rhs)
    dnat_bf_d = din("dnat_bf", (2, 2, 128, 1024), BF16)  # [v][th][j%128, jcl*512+i]
    dneg_d = din("dneg", (2, 2, 128, 1024), BF16)  # [v][th][i%128, i2*512+j]
    dtm_d = din("dtm", (2, 128, 1024), BF16)     # [th][i%128, i2*512+j] (v=0 mask)
    eye_bf_d = din("eye_bf", (128, 128), BF16)
    wk_d = din("wk", (L, D, H * HS))
    wq_d = din("wq", (L, D, H * HS))
    wv_d = din("wv", (L, D, H * HS))
    wp_d = din("wp", (L, H * HS, D))
    w1_d = din("w1", (L, D, FF))
    w2_d = din("w2_bf", (L, FF, D), BF16)
    wlm_d = din("wlm", (D, D))
    out_d = nc.dram_tensor("out", [BPC, D, N], F32, kind="ExternalOutput").ap()

    with tile.TileContext(nc) as tc, ExitStack() as ctx:
        # ---------------- pools ----------------
        pconst = ctx.enter_context(tc.tile_pool(name="pconst", bufs=1))
        pw = ctx.enter_context(tc.tile_pool(name="pw", bufs=2))      # kqv weights
        pw1 = ctx.enter_context(tc.tile_pool(name="pw1", bufs=1))    # wp/w1/w2
        pxt = ctx.enter_context(tc.tile_pool(name="pxt", bufs=2))
        pkqv = ctx.enter_context(tc.tile_pool(name="pkqv", bufs=1))  # k/q/v per b
        pqd = ctx.enter_context(tc.tile_pool(name="pqd", bufs=2))
        pem = ctx.enter_context(tc.tile_pool(name="pem", bufs=3))
        psml = ctx.enter_context(tc.tile_pool(name="psml", bufs=3))
        pspm = ctx.enter_context(tc.tile_pool(name="pspm", bufs=2))
        pptsb = ctx.enter_context(tc.tile_pool(name="pptsb", bufs=2))
        po = ctx.enter_context(tc.tile_pool(name="po", bufs=1))
        pmha = ctx.enter_context(tc.tile_pool(name="pmha", bufs=1))
        pff1 = ctx.enter_context(tc.tile_pool(name="pff1", bufs=1))
        pout = ctx.enter_context(tc.tile_pool(name="pout", bufs=1))
        # PSUM: 4 + 2 + 2 = 8 banks
        ps_big = ctx.enter_context(tc.tile_pool(name="ps_big", bufs=2, space="PSUM"))
        ps_pt = ctx.enter_context(tc.tile_pool(name="ps_pt", bufs=2, space="PSUM"))
        ps_sm = ctx.enter_context(tc.tile_pool(name="ps_sm", bufs=2, space="PSUM"))

        # ---------------- static loads ----------------
        eye_bf = pconst.tile([128, 128], BF16, tag="eye", name="eye")
        nc.sync.dma_start(eye_bf[:], eye_bf_d[:])
        l0b = pconst.tile([128, 1], F32, tag="l0b", name="l0b")
        nc.vector.memset(l0b[:], -L0_SHIFT)

        d_r = {}      # [(v, mc)] (128, 512) f32r: d[m, i], rows m-chunk
        dnat = {}     # [(v, th)] (128, 1024) bf16: rows j%128, [jcl*512+i]
        dng = {}
        dtm = {}      # [(v, ic)] (128, 512) f32: additive mask rows i-chunk
        for v in range(2):
            for c in range(NC4):
                t = pconst.tile([128, N], F32R, tag=f"d{v}_{c}", name=f"d{v}_{c}")
                nc.sync.dma_start(t[:], dmat_d[v, c * 128:(c + 1) * 128, :]
                                  .bitcast(F32R))
                d_r[(v, c)] = t

            for th in range(2):
                tn = pconst.tile([128, 1024], BF16, tag=f"dn{v}_{th}",
                                 name=f"dn{v}_{th}")
                nc.sync.dma_start(tn[:], dnat_bf_d[v, th])
                dnat[(v, th)] = tn
                tg = pconst.tile([128, 1024], BF16, tag=f"dg{v}_{th}",
                                 name=f"dg{v}_{th}")
                nc.sync.dma_start(tg[:], dneg_d[v, th])
                dng[(v, th)] = tg
                if v == 0:
                    tq = pconst.tile([128, 1024], BF16, tag=f"dtm{th}",
                                     name=f"dtm{th}")
                    nc.sync.dma_start(tq[:], dtm_d[th])
                    dtm[(0, th)] = tq

        wlm_t = []
        for kc in range(DC):
            t = pconst.tile([128, D], F32R, tag=f"wlm{kc}", name=f"wlm{kc}")
            nc.sync.dma_start(t[:], wlm_d[kc * 128:(kc + 1) * 128, :]
                              .bitcast(F32R))
            wlm_t.append(t)

        xt_cur = {}
        for b in range(BPC):
            tiles = []
            for c in range(DC):
                t = pxt.tile([128, N], F32R, tag=f"xt{b}_{c}", name=f"xt{b}_{c}")
                nc.sync.dma_start(t[:], xt_d[b, c * 128:(c + 1) * 128, :]
                                  .bitcast(F32R))
                tiles.append(t)
            xt_cur[b] = tiles

        # ---------------- layers ----------------
        for l in range(L):
            v = 0 if l == 0 else 1

            wk_t, wq_t, wv_t = [], [], []
            for kc in range(DC):
                for (dst, src, nm) in ((wk_t, wk_d, "wk"), (wq_t, wq_d, "wq"),
                                       (wv_t, wv_d, "wv")):
                    t = pw.tile([128, H * HS], F32R, tag=f"{nm}{kc}", name=nm)
                    nc.sync.dma_start(t[:], src[l, kc * 128:(kc + 1) * 128, :]
                                      .bitcast(F32R))
                    dst.append(t)
            wp_t = []
            for kc in range(4):
                t = pw1.tile([128, D], F32R, tag=f"wp{kc}", name="wp")
                nc.sync.dma_start(t[:], wp_d[l, kc * 128:(kc + 1) * 128, :]
                                  .bitcast(F32R))
                wp_t.append(t)
            w1_t = []
            for kc in range(DC):
                t = pw1.tile([128, FF], F32R, tag=f"w1{kc}", name="w1")
                nc.sync.dma_start(t[:], w1_d[l, kc * 128:(kc + 1) * 128, :]
                                  .bitcast(F32R))
                w1_t.append(t)
            w2_t = []
            for kc in range(FC):
                t = pw1.tile([128, D], BF16, tag=f"w2{kc}", name="w2")
                nc.sync.dma_start(t[:], w2_d[l, kc * 128:(kc + 1) * 128, :])
                w2_t.append(t)

            # ---- KQV phase (both b) ----
            k_sb, q_nat, v_nat = {}, {}, {}
            for b in range(BPC):
                xt = xt_cur[b]
                ks, qs, vs = [], [], []
                for g in range(2):
                    # K pair-packed: (128=2heads*64, m); hp = 2g, 2g+1
                    mm = ps_big.tile([128, 1024], F32, tag="big", name="kps")
                    for i2 in range(2):
                        hp = 2 * g + i2
                        for kc in range(DC):
                            nc.tensor.matmul(
                                mm[:, i2 * 512:(i2 + 1) * 512],
                                wk_t[kc][:, hp * 128:(hp + 1) * 128],
                                xt[kc][:], start=(kc == 0), stop=(kc == DC - 1))
                    sb = pkqv.tile([128, 1024], F32R, tag=f"k{b}{g}", name="ksb")
                    nc.scalar.activation(sb[:], mm[:], AFT.Silu)
                    ks.append(sb)
                for g in range(2):
                    # Q natural: (m-chunk, hk) via lhsT = xt;  mc = 2g, 2g+1
                    mm = ps_big.tile([128, 1024], F32, tag="big", name="qps")
                    for i2 in range(2):
                        mc = 2 * g + i2
                        for kc in range(DC):
                            nc.tensor.matmul(
                                mm[:, i2 * 512:(i2 + 1) * 512],
                                xt[kc][:, mc * 128:(mc + 1) * 128],
                                wq_t[kc][:], start=(kc == 0), stop=(kc == DC - 1))
                    sb = pkqv.tile([128, 1024], F32R, tag=f"q{b}{g}", name="qsb")
                    nc.scalar.activation(sb[:], mm[:], AFT.Silu)
                    qs.append(sb)
                for g in range(2):
                    # V natural (j-chunk, hk), bf16
                    mm = ps_big.tile([128, 1024], F32, tag="big", name="vps")
                    for i2 in range(2):
                        jc = 2 * g + i2
                        for kc in range(DC):
                            nc.tensor.matmul(
                                mm[:, i2 * 512:(i2 + 1) * 512],
                                xt[kc][:, jc * 128:(jc + 1) * 128],
                                wv_t[kc][:], start=(kc == 0), stop=(kc == DC - 1))
                    sb = pkqv.tile([128, 1024], BF16, tag=f"v{b}{g}", name="vsb")
                    nc.scalar.activation(sb[:], mm[:], AFT.Silu)
                    vs.append(sb)
                k_sb[b], q_nat[b], v_nat[b] = ks, qs, vs

            # ---- head loop: b-interleaved, software-pipelined ----
            seq = [(b, h) for h in range(H) for b in range(BPC)]
            qd_sb = {}
            o_all = {b: [None] * 4 for b in range(BPC)}
            o_ps = {}
            state = {}
            ptadd_ctr = [0]

            def emit_qd(b, hp):
                mm = ps_sm.tile([128, N], F32, tag="sm", name="qdps")
                for mc in range(NC4):
                    g, i2 = mc // 2, mc % 2
                    nc.tensor.matmul(
                        mm[:],
                        q_nat[b][g][:, i2 * 512 + hp * 128:
                                    i2 * 512 + (hp + 1) * 128],
                        d_r[(v, mc)][:], start=(mc == 0), stop=(mc == NC4 - 1))
                sb = pqd.tile([128, N], F32R, tag=f"qd{b}", name="qdsb")
                nc.scalar.activation(sb[:], mm[:], AFT.Copy)
                qd_sb[b] = sb

            def stage_front(b, h):
                hp, half = h // 2, h % 2
                lo, hi = half * 64, (half + 1) * 64
                qd = qd_sb[b]
                kg, ki = hp // 2, hp % 2
                ksl = k_sb[b][kg]
                # T matmuls -> 2 psum tiles (128,1024) = [ic0|ic1], [ic2|ic3]
                tts = []
                for th in range(2):
                    tt = ps_big.tile([128, 1024], F32, tag="big", name="tps")
                    for i2 in range(2):
                        ic = 2 * th + i2
                        nc.tensor.matmul(
                            tt[:, i2 * 512:(i2 + 1) * 512],
                            qd[lo:hi, ic * 128:(ic + 1) * 128],
                            ksl[lo:hi, ki * 512:(ki + 1) * 512],
                            start=True, stop=True)
                    tts.append(tt)
                em = pem.tile([128, 2048], BF16, tag="em", name="em")
                ssum = psml.tile([128, 4], F32, tag="ssum", name="ssum")
                s4 = psml.tile([128, 4], F32, tag="s4", name="s4")
                if l == 0 and L0_FAST:
                    # layer-0: logits bounded (|spm| <= 143 for this data):
                    # keep the mask stt, skip the rowmax reduce, use a
                    # constant exp shift; dead rows via ssum<=0.
                    for ic in range(NC4):
                        th, i2 = ic // 2, ic % 2
                        spm = pspm.tile([128, 512], F32, tag=f"spm{ic % 2}",
                                        name="spm", bufs=2)
                        nc.vector.scalar_tensor_tensor(
                            spm[:, 0:512],
                            tts[th][:, i2 * 512:(i2 + 1) * 512],
                            0.125, dng[(v, th)][:, i2 * 512:(i2 + 1) * 512],
                            ALU.mult, ALU.add)
                        nc.scalar.activation(
                            em[:, ic * 512:(ic + 1) * 512],
                            spm[:, 0:512], AFT.Exp,
                            bias=l0b[:, 0:1], scale=1.0,
                            accum_out=ssum[:, ic:ic + 1])
                    al = psml.tile([128, 4], F32, tag="al", name="al")
                    nc.vector.tensor_scalar(al[:], ssum[:], 0.0, None,
                                            ALU.is_le)
                    nc.vector.tensor_tensor(al[:], ssum[:], al[:], ALU.add)
                    nc.vector.reciprocal(s4[:], al[:])
                else:
                    negm = psml.tile([128, 4], F32, tag="negm", name="negm")
                    al = psml.tile([128, 4], F32, tag="al", name="al")
                    for ic in range(NC4):
                        th, i2 = ic // 2, ic % 2
                        spm = pspm.tile([128, 512], F32, tag=f"spm{ic % 2}",
                                        name="spm", bufs=2)
                        nc.vector.scalar_tensor_tensor(
                            spm[:, 0:512],
                            tts[th][:, i2 * 512:(i2 + 1) * 512],
                            0.125, dng[(v, th)][:, i2 * 512:(i2 + 1) * 512],
                            ALU.mult, ALU.add)
                        nc.vector.tensor_reduce(
                            negm[:, ic:ic + 1], spm[:, 0:512], AX.X,
                            ALU.max, negate=True)
                        nc.scalar.activation(
                            em[:, ic * 512:(ic + 1) * 512],
                            spm[:, 0:512], AFT.Exp,
                            bias=negm[:, ic:ic + 1], scale=1.0,
                            accum_out=ssum[:, ic:ic + 1])
                    nc.vector.tensor_scalar(al[:], negm[:], 1.0e28,
                                            None, ALU.is_lt)
                    nc.vector.reciprocal(s4[:], ssum[:])
                    nc.vector.tensor_tensor(s4[:], s4[:], al[:], ALU.mult)
                for ic in range(NC4):
                    sl = em[:, ic * 512:(ic + 1) * 512]
                    if ic < 2:
                        nc.vector.tensor_scalar(sl, sl, s4[:, ic:ic + 1],
                                                None, ALU.mult)
                    else:
                        nc.scalar.activation(sl, sl, AFT.Copy, bias=0.0,
                                             scale=s4[:, ic:ic + 1])
                state[(b, h)] = (em,)

            def stage_back(b, h):
                hp, half = h // 2, h % 2
                (em,) = state.pop((b, h))
                # transposes (plain identity) -> pt psum (j, i) bf16
                pts = []
                for th in range(2):
                    pts.append(ps_pt.tile([128, 1024], BF16, tag="pt", name="pt"))
                for ic in range(NC4):
                    for jc in range(NC4):
                        nc.tensor.transpose(
                            pts[jc // 2][:, (jc % 2) * 512 + ic * 128:
                                         (jc % 2) * 512 + (ic + 1) * 128],
                            em[:, ic * 512 + jc * 128: ic * 512 + (jc + 1) * 128],
                            eye_bf[:])
                # Ppl^T = pt + d  (psum->sbuf on DVE)
                ptsb = []
                for th in range(2):
                    t = pptsb.tile([128, 1024], BF16, tag=f"ptsb{th}", name="ptsb")
                    nc.vector.tensor_tensor(t[:], pts[th][:], dnat[(v, th)][:],
                                            ALU.add)
                    ptsb.append(t)
                # O = Ppl @ V : accumulate into pair-packed psum
                if half == 0:
                    o_ps[b] = ps_sm.tile([128, N], F32, tag="sm", name="ops")
                lo = half * 64
                vg = v_nat[b]
                for jc in range(NC4):
                    nc.tensor.matmul(
                        o_ps[b][lo:lo + 64, :],
                        vg[jc // 2][:, (jc % 2) * 512 + h * 64:
                                    (jc % 2) * 512 + (h + 1) * 64],
                        ptsb[jc // 2][:, (jc % 2) * 512:(jc % 2 + 1) * 512],
                        start=(jc == 0), stop=(jc == NC4 - 1))
                if half == 1:
                    ot = po.tile([128, N], F32R, tag=f"o{b}{hp}", name="oall")
                    nc.scalar.activation(ot[:], o_ps[b][:], AFT.Copy)
                    o_all[b][hp] = ot

            for i in range(len(seq) + LOOKAHEAD):
                if i >= LOOKAHEAD:
                    stage_back(*seq[i - LOOKAHEAD])
                if i < len(seq):
                    b, h = seq[i]
                    if h % 2 == 0:
                        emit_qd(b, h // 2)
                    stage_front(b, h)

            # ---- MLP (per b) ----
            for b in range(BPC):
                mm = ps_big.tile([128, 1024], F32, tag="big", name="mhaps")
                for mc in range(DC):
                    for kc in range(4):
                        nc.tensor.matmul(
                            mm[:, mc * 512:(mc + 1) * 512],
                            wp_t[kc][:, mc * 128:(mc + 1) * 128],
                            o_all[b][kc][:], start=(kc == 0), stop=(kc == 3))
                mha = pmha.tile([128, 1024], F32R, tag=f"mha{b}", name="mha")
                nc.scalar.activation(mha[:], mm[:], AFT.Silu)

                ff1 = []
                for g in range(4):  # fc pairs
                    mm = ps_big.tile([128, 1024], F32, tag="big", name="ff1ps")
                    for i2 in range(2):
                        fc = 2 * g + i2
                        for mc in range(DC):
                            nc.tensor.matmul(
                                mm[:, i2 * 512:(i2 + 1) * 512],
                                w1_t[mc][:, fc * 128:(fc + 1) * 128],
                                mha[:, mc * 512:(mc + 1) * 512],
                                start=(mc == 0), stop=(mc == DC - 1))
                    t = pff1.tile([128, 1024], BF16, tag=f"ff1{g}", name="ff1")
                    nc.scalar.activation(t[:], mm[:], AFT.Silu)
                    ff1.append(t)

                xt_new = []
                for mc in range(DC):
                    mm = ps_sm.tile([128, N], F32, tag="sm", name="ff2ps")
                    for fc in range(FC):
                        nc.tensor.matmul(
                            mm[:], w2_t[fc][:, mc * 128:(mc + 1) * 128],
                            ff1[fc // 2][:, (fc % 2) * 512:(fc % 2 + 1) * 512],
                            start=(fc == 0), stop=(fc == FC - 1))
                    t = pxt.tile([128, N], F32R, tag=f"xt{b}_{mc}",
                                 name=f"xt{b}_{mc}")
                    nc.vector.tensor_tensor(t[:], mm[:],
                                            mha[:, mc * 512:(mc + 1) * 512],
                                            ALU.add)
                    xt_new.append(t)
                xt_cur[b] = xt_new

        # ---------------- lm head ----------------
        for b in range(BPC):
            for mc in range(DC):
                mm = ps_sm.tile([128, N], F32, tag="sm", name="lmps")
                for kc in range(DC):
                    nc.tensor.matmul(
                        mm[:], wlm_t[kc][:, mc * 128:(mc + 1) * 128],
                        xt_cur[b][kc][:], start=(kc == 0), stop=(kc == DC - 1))
                ot = pout.tile([128, N], F32, tag="out", name="out")
                nc.scalar.activation(ot[:], mm[:], AFT.Copy)
                nc.sync.dma_start(out_d[b, mc * 128:(mc + 1) * 128, :], ot[:])

    nc.compile()
    return nc


_NC_CACHE = None


def _get_nc():
    global _NC_CACHE
    if _NC_CACHE is None:
        _NC_CACHE = _build()
    return _NC_CACHE


def _bf16_np(x):
    import ml_dtypes
    return np.ascontiguousarray(
        np.asarray(x, dtype=np.float32).astype(ml_dtypes.bfloat16))


def _prep_inputs(inputs):
    f = lambda x: np.ascontiguousarray(np.asarray(x, dtype=np.float32))
    for bn in ("bk", "bq", "bv", "bp", "b1", "b2", "blm"):
        if np.any(np.asarray(inputs[bn]) != 0):
            raise ValueError(f"kernel compiled for zero {bn}")
    X = f(inputs["X"])
    dag = np.asarray(inputs["dag"])
    d0 = np.clip(dag.astype(np.float32), 0.0, 1.0)
    d1 = np.clip(d0 + np.eye(N, dtype=np.float32), 0.0, 1.0)
    dmat = np.stack([d0, d1])                          # [v][m, i]
    # [v][th][j%128, jcl*512 + i]
    dnat_bf = _bf16_np(dmat.reshape(2, 2, 2, 128, N).transpose(0, 1, 3, 2, 4)
                       .reshape(2, 2, 128, 1024))
    dnegf = (dmat.transpose(0, 2, 1) - 1.0) * NEG_BIG      # [v][i, j]
    dneg = _bf16_np(dnegf.reshape(2, 2, 2, 128, N).transpose(0, 1, 3, 2, 4)
                    .reshape(2, 2, 128, 1024))
    dtm = _bf16_np(dmat[0].T.reshape(2, 2, 128, N).transpose(0, 2, 1, 3)
                   .reshape(2, 128, 1024))
    wr = lambda w: np.ascontiguousarray(
        f(w).transpose(0, 2, 1, 3).reshape(L, D, H * HS))
    common = {
        "dmat": np.ascontiguousarray(dmat),
        "dnat_bf": dnat_bf,
        "dneg": dneg,
        "dtm": dtm,
        "eye_bf": _bf16_np(np.eye(128, dtype=np.float32)),
        "wk": wr(inputs["Wk"]), "wq": wr(inputs["Wq"]), "wv": wr(inputs["Wv"]),
        "wp": f(inputs["Wp"]),
        "w1": f(inputs["W1"]),
        "w2_bf": _bf16_np(inputs["W2"]),
        "wlm": f(inputs["Wlm"]),
    }
    xt_full = np.ascontiguousarray(X.transpose(0, 2, 1))   # (B, D, N)
    in_maps = []
    for c in range(NCORES):
        m = dict(common)
        m["xt"] = np.ascontiguousarray(xt_full[c * BPC:(c + 1) * BPC])
        in_maps.append(m)
    return in_maps


def run(inputs, trace=False):
    from concourse.bass_utils import run_bass_kernel_spmd

    if trace:
        _install_ntff_hook()
    nc = _get_nc()
    in_maps = _prep_inputs(inputs)
    res = run_bass_kernel_spmd(nc, in_maps, list(range(NCORES)), trace=trace)
    outs = np.concatenate([res.results[c]["out"] for c in range(NCORES)], 0)
    full = np.ascontiguousarray(outs.transpose(0, 2, 1).astype(np.float32))
    return full, res


def kernel(**inputs):
    out, _ = run(inputs, trace=False)
    return out


if __name__ == "__main__":
    rng = np.random.default_rng(0)
    fake = {
        "X": rng.standard_normal((B, N, D), dtype=np.float32),
        "dag": rng.integers(0, 2, (N, N)).astype(np.int32),
        "Wk": rng.standard_normal((L, H, D, HS), dtype=np.float32) * 0.05,
        "bk": np.zeros((L, H, HS), np.float32),
        "Wq": rng.standard_normal((L, H, D, HS), dtype=np.float32) * 0.05,
        "bq": np.zeros((L, H, HS), np.float32),
        "Wv": rng.standard_normal((L, H, D, HS), dtype=np.float32) * 0.05,
        "bv": np.zeros((L, H, HS), np.float32),
        "Wp": rng.standard_normal((L, H * HS, D), dtype=np.float32) * 0.05,
        "bp": np.zeros((L, D), np.float32),
        "W1": rng.standard_normal((L, D, FF), dtype=np.float32) * 0.05,
        "b1": np.zeros((L, FF), np.float32),
        "W2": rng.standard_normal((L, FF, D), dtype=np.float32) * 0.05,
        "b2": np.zeros((L, D), np.float32),
        "Wlm": rng.standard_normal((D, D), dtype=np.float32) * 0.05,
        "blm": np.zeros((D,), np.float32),
    }
    out = kernel(**fake)
    print("out", out.shape, out.dtype, np.abs(out).mean())
